# Initial kernel scaffold
#
"""Distributed Trainium2 kernel for AttributeHypergraphModel (2x GATConv over
triples with attribute-attention entity embeddings).

Strategy (8 NeuronCores, SPMD):
  - nodes are relabeled on the host: sorted by (in-degree, A-side edge count)
    and dealt round-robin to cores, so every core's tile t has near-identical
    padded shapes (required: one SPMD graph) and padded gather groups waste
    little traffic.
  - attr/rel tables are projected once on device (matmul); the projected attr
    table is sharded + AllGathered. Entity-embedding attention and both GAT
    layers then run on dma_gather'ed rows (A/B split tables keep gather
    indices under the signed-int16 ucode limit; -1e30 mask planes neutralize
    padding slots).
  - each GAT layer: dense matmul with folded alpha_dst column, AllGather of
    node features, dst-partitioned softmax + weighted sum per 128-dst group.
All index/mask planes are precomputed host-side; outputs are un-permuted on
the host.
"""

import sys

sys.path.insert(0, "/opt/trn_rl_repo")

import numpy as np

NCORE = 8
N = 50000
A = 16
NREL = 500
DE = 128
NPAD = 6272  # 49 tiles of 128 local slots per core
NTILE = NPAD // 128
NTOT = NPAD * NCORE  # 50176 global slots
SHARD = N // NCORE  # 6250 real rows per core (attr table + nodes)
SPLIT = 32768
NEGB = np.float32(-1.0e30)
NEG_SLOPE = 0.2


# ---------------------------------------------------------------- planning --


def _pack_idx(plane):
    """[128, c] int plane (slot p gets column j at gather position j*128+p)
    -> int16 SBUF index layout [128, 8*c] (16-row pattern replicated x8)."""
    p128, c = plane.shape
    assert p128 == 128
    assert plane.min(initial=0) >= 0 and plane.max(initial=0) < 32768
    vals = plane.T.reshape(-1)  # logical gather order
    cols = vals.size // 16
    arr = vals.reshape(cols, 16).T  # arr[i%16, i//16] = vals[i]
    return np.ascontiguousarray(np.tile(arr, (8, 1)).astype(np.int16))


def _column_planes(padded, k_a, total, c_a, c_b, split):
    """Split per-row id lists (A-first order in `padded`) into A/B column
    planes plus additive mask biases (-1e30 on padding)."""
    colA = np.arange(c_a)[None, :]
    mA = colA < k_a[:, None]
    pA = np.where(mA, padded[:, :c_a], 0).astype(np.int64)
    bA = np.where(mA, np.float32(0), NEGB).astype(np.float32)
    colB = np.arange(c_b)[None, :]
    mB = colB < (total - k_a)[:, None]
    gidx = np.minimum(k_a[:, None] + colB, padded.shape[1] - 1)
    pB = np.where(mB, np.take_along_axis(padded, gidx, axis=1) - split, 0)
    pB = pB.astype(np.int64)
    bB = np.where(mB, np.float32(0), NEGB).astype(np.float32)
    return pA, bA, pB, bB


def _build_family(ordered, kA, total, cA, cB, split):
    """ordered: [NCORE*NPAD, W] id lists (A ids first); returns per-tile
    cA/cB and per-core concatenated idx/mask planes."""
    nrow = ordered.shape[0]
    per_core = nrow // NCORE
    ntile = per_core // 128
    idx_a = [[] for _ in range(NCORE)]
    idx_b = [[] for _ in range(NCORE)]
    masks = [[] for _ in range(NCORE)]
    for c in range(NCORE):
        for t in range(ntile):
            r0 = c * per_core + t * 128
            pA, bA, pB, bB = _column_planes(
                ordered[r0 : r0 + 128], kA[r0 : r0 + 128], total[r0 : r0 + 128],
                int(cA[t]), int(cB[t]), split,
            )
            idx_a[c].append(_pack_idx(pA))
            idx_b[c].append(_pack_idx(pB))
            masks[c].append(np.concatenate([bA, bB], axis=1))
    return dict(
        cA=[int(x) for x in cA],
        cB=[int(x) for x in cB],
        idxA=[np.ascontiguousarray(np.concatenate(v, axis=1)) for v in idx_a],
        idxB=[np.ascontiguousarray(np.concatenate(v, axis=1)) for v in idx_b],
        mask=[np.ascontiguousarray(np.concatenate(v, axis=1)) for v in masks],
    )


def _family_from_lists(ids, valid, split):
    """ids: [NCORE*NPAD, A] raw ids (already in table-slot space), valid rows
    marked; builds A-first ordering then the family planes."""
    ids = np.where(ids < 0, 0, ids)
    isB = ids >= split
    perm = np.argsort(isB, axis=1, kind="stable")
    ordered = np.take_along_axis(ids, perm, axis=1)
    kA = (~isB).sum(axis=1).astype(np.int64)
    total = np.full(len(ids), ids.shape[1], np.int64)
    kA[~valid] = 0
    total[~valid] = 0
    ordered = np.concatenate([ordered, np.zeros_like(ordered)], axis=1)
    kA3 = kA.reshape(NCORE, NTILE, 128)
    tot3 = total.reshape(NCORE, NTILE, 128)
    cA = np.maximum(kA3.max(axis=(0, 2)), 1)
    cB = np.maximum((tot3 - kA3).max(axis=(0, 2)), 1)
    return _build_family(ordered, kA, total, cA, cB, split)


def _remap_attr(ids):
    """raw attr id -> row in the padded AllGather'ed projection table."""
    return (ids // SHARD) * NPAD + (ids % SHARD)


def make_plan(h_attributes, t_attributes, r_idx, edge_index):
    h_attributes = np.asarray(h_attributes)
    t_attributes = np.asarray(t_attributes)
    r_idx = np.asarray(r_idx)
    edge_index = np.asarray(edge_index)

    src0 = np.concatenate([edge_index[0], np.arange(N, dtype=np.int64)])
    dst0 = np.concatenate([edge_index[1], np.arange(N, dtype=np.int64)])
    deg = np.bincount(dst0, minlength=N)

    def slots_from_order(order):
        rank = np.empty(N, np.int64)
        rank[order] = np.arange(N)
        core_of = rank % NCORE
        local_of = rank // NCORE
        return core_of * NPAD + local_of, core_of, local_of

    g0, _, _ = slots_from_order(np.argsort(deg, kind="stable"))
    kAe0 = np.bincount(dst0[g0[src0] < SPLIT], minlength=N)
    order = np.lexsort((kAe0, deg))
    gslot, core_of, local_of = slots_from_order(order)

    # ---- attr families (ids remapped into padded projection-table space)
    attrs_h = np.full((NCORE * NPAD, A), -1, np.int64)
    attrs_t = np.full((NCORE * NPAD, A), -1, np.int64)
    valid = np.zeros(NCORE * NPAD, bool)
    attrs_h[gslot] = _remap_attr(h_attributes)
    attrs_t[gslot] = _remap_attr(t_attributes)
    valid[gslot] = True
    fam_h = _family_from_lists(attrs_h, valid, SPLIT)
    fam_t = _family_from_lists(attrs_t, valid, SPLIT)

    # ---- r_idx gather planes
    r_slot = np.zeros(NCORE * NPAD, np.int64)
    r_slot[gslot] = r_idx
    r_slot = r_slot.reshape(NCORE, NPAD)
    ridx_planes = []
    for c in range(NCORE):
        cols = [_pack_idx(r_slot[c, t * 128 : (t + 1) * 128][:, None])
                for t in range(NTILE)]
        ridx_planes.append(np.ascontiguousarray(np.concatenate(cols, axis=1)))

    # ---- edge family (per-dst in-edge src slots, A-first)
    sg = gslot[src0]
    dg = gslot[dst0]
    order_e = np.lexsort(((sg >= SPLIT).astype(np.int64), dg))
    sg_s = sg[order_e]
    dg_s = dg[order_e]
    cnt = np.bincount(dg_s, minlength=NTOT)
    starts = np.concatenate([[0], np.cumsum(cnt)[:-1]])
    pos = np.arange(len(sg_s)) - starts[dg_s]
    maxdeg = int(cnt.max())
    padded_e = np.zeros((NTOT, maxdeg + 8), np.int64)
    padded_e[dg_s, pos] = sg_s
    kAe = np.bincount(dg_s[sg_s < SPLIT], minlength=NTOT).astype(np.int64)
    tot_e = cnt.astype(np.int64)
    kA3 = kAe.reshape(NCORE, NTILE, 128)
    tot3 = tot_e.reshape(NCORE, NTILE, 128)
    cAe = np.maximum(kA3.max(axis=(0, 2)), 1)
    cBe = np.maximum((tot3 - kA3).max(axis=(0, 2)), 1)
    need = int(cAe.max() + cBe.max())
    if padded_e.shape[1] < need:
        padded_e = np.concatenate(
            [padded_e, np.zeros((NTOT, need - padded_e.shape[1]), np.int64)],
            axis=1)
    fam_e = _build_family(padded_e, kAe, tot_e, cAe, cBe, SPLIT)

    return dict(core_of=core_of, local_of=local_of,
                fam_h=fam_h, fam_t=fam_t, fam_e=fam_e, ridx=ridx_planes)


def make_weights(attr_table, rel_table, femb_w, femb_b,
                 gat1_w, gat1_asrc, gat1_adst, gat1_b,
                 gat2_w, gat2_asrc, gat2_adst, gat2_b):
    f32 = np.float32
    w = {}
    w["attr_tT"] = np.ascontiguousarray(np.asarray(attr_table, f32).T)
    w["rel_tT"] = np.ascontiguousarray(np.asarray(rel_table, f32).T)
    w["rel_rows"] = np.ascontiguousarray(np.asarray(rel_table, f32))
    w["femb_wt"] = np.ascontiguousarray(np.asarray(femb_w, f32).T)
    w["femb_b_rep"] = np.ascontiguousarray(
        np.tile(np.asarray(femb_b, f32)[None, :], (128, 1)))
    for i, (gw, gas, gad, gb) in enumerate(
        [(gat1_w, gat1_asrc, gat1_adst, gat1_b),
         (gat2_w, gat2_asrc, gat2_adst, gat2_b)], start=1
    ):
        gw = np.asarray(gw, f32)
        aug = np.concatenate(
            [gw.T, (gw.T @ np.asarray(gas, f32))[:, None],
             (gw.T @ np.asarray(gad, f32))[:, None]], axis=1)
        w[f"waug{i}"] = np.ascontiguousarray(aug)  # [Din, 130]
        w[f"asrc{i}_rep"] = np.ascontiguousarray(
            np.tile(np.asarray(gas, f32)[None, :], (128, 1)))
        w[f"b{i}_rep"] = np.ascontiguousarray(
            np.tile(np.asarray(gb, f32)[None, :], (128, 1)))
    w["ident"] = np.eye(128, dtype=f32)
    return w


# ------------------------------------------------------- numpy device model --


def _sim_gather(table, idx_packed, num, elem):
    arr = idx_packed[:16]
    vals = arr.T.reshape(-1)[:num].astype(np.int64)
    rows = table[vals]
    return rows.reshape(num // 128, 128, elem).transpose(1, 0, 2)


def _fam_off(fam, t):
    oA = 8 * sum(fam["cA"][:t])
    oB = 8 * sum(fam["cB"][:t])
    oM = sum(fam["cA"][i] + fam["cB"][i] for i in range(t))
    return oA, oB, oM


def simulate(plan, weights, inputs):
    """Numpy mirror of the device program (validates the planner)."""
    f32 = np.float32
    attr_proj = (np.asarray(inputs["attr_table"], f32) @ weights["femb_wt"]
                 + weights["femb_b_rep"][0])
    proj_pad = np.zeros((NTOT, DE), f32)
    for c in range(NCORE):
        proj_pad[c * NPAD : c * NPAD + SHARD] = \
            attr_proj[c * SHARD : (c + 1) * SHARD]
    rel_proj = (np.asarray(inputs["rel_table"], f32) @ weights["femb_wt"]
                + weights["femb_b_rep"][0])
    rel_comb = np.concatenate([rel_proj, weights["rel_rows"]], axis=1)
    tab_A, tab_B = proj_pad[:SPLIT], proj_pad[SPLIT:]

    def softmax_gather(tabA, tabB, fam, core, t, query_fn, extra=None,
                       lrelu=False):
        cA, cB = fam["cA"][t], fam["cB"][t]
        oA, oB, oM = _fam_off(fam, t)
        gA = _sim_gather(tabA, fam["idxA"][core][:, oA : oA + 8 * cA],
                         128 * cA, DE)
        gB = _sim_gather(tabB, fam["idxB"][core][:, oB : oB + 8 * cB],
                         128 * cB, DE)
        mask = fam["mask"][core][:, oM : oM + cA + cB]
        G = np.concatenate([gA, gB], axis=1)
        s = (G * query_fn()).sum(-1)
        if extra is not None:
            s = s + extra
        s = s + mask
        if lrelu:
            s = np.maximum(s, NEG_SLOPE * s)
        m = s.max(axis=1, keepdims=True)
        ex = np.exp(s - m)
        return (G * ex[:, :, None]).sum(axis=1) / ex.sum(axis=1, keepdims=True)

    triple = np.zeros((NCORE, NPAD, 256), f32)
    for c in range(NCORE):
        for t in range(NTILE):
            rid = plan["ridx"][c][:16, 8 * t : 8 * t + 8].T.reshape(-1)[:128]
            rc = rel_comb[rid.astype(np.int64)]
            rp, re = rc[:, :128], rc[:, 128:]
            he = softmax_gather(tab_A, tab_B, plan["fam_h"], c, t,
                                lambda: rp[:, None, :])
            te = softmax_gather(tab_A, tab_B, plan["fam_t"], c, t,
                                lambda: rp[:, None, :])
            triple[c, t * 128 : (t + 1) * 128, :128] = he + te
            triple[c, t * 128 : (t + 1) * 128, 128:] = re

    def gat(x_all, waug, asrc_rep, b_rep, layer):
        h = x_all.reshape(NTOT, -1) @ waug
        hF = np.ascontiguousarray(h[:, :128])
        ad = h[:, 129]
        out = np.zeros((NCORE, NPAD, 128), f32)
        fam = plan["fam_e"]
        for c in range(NCORE):
            for g_i in range(NTILE):
                sl = slice(c * NPAD + g_i * 128, c * NPAD + (g_i + 1) * 128)
                agg = softmax_gather(
                    hF[:SPLIT], hF[SPLIT:], fam, c, g_i,
                    lambda: asrc_rep[0][None, None, :],
                    extra=ad[sl][:, None], lrelu=True)
                out[c, g_i * 128 : (g_i + 1) * 128] = agg + b_rep[0][None, :]
        return out

    x1 = gat(triple, weights["waug1"], weights["asrc1_rep"],
             weights["b1_rep"], 1)
    x2 = gat(x1, weights["waug2"], weights["asrc2_rep"], weights["b2_rep"], 2)
    return x2.reshape(NCORE, NPAD, 128)[plan["core_of"], plan["local_of"]]


# ------------------------------------------------------------ bass program --


def build_bass(plan):
    import copy as _copy
    import concourse.bass as bass
    import concourse.bacc as bacc
    import concourse.mybir as mb
    from contextlib import ExitStack

    F32 = mb.dt.float32
    I16 = mb.dt.int16
    fam_h, fam_t, fam_e = plan["fam_h"], plan["fam_t"], plan["fam_e"]

    nc = bacc.Bacc(target_bir_lowering=False, debug=True)

    def par(name, shape, dt=F32, out=False):
        return nc.declare_dram_parameter(name, list(shape), dt, isOutput=out)

    attr_tT = par("attr_tT", [128, N])
    rel_tT = par("rel_tT", [128, NREL])
    rel_rows = par("rel_rows", [NREL, 128])
    femb_wt = par("femb_wt", [128, 128])
    femb_b_rep = par("femb_b_rep", [128, 128])
    waug1 = par("waug1", [256, 130])
    waug2 = par("waug2", [128, 130])
    asrc1_rep = par("asrc1_rep", [128, 128])
    asrc2_rep = par("asrc2_rep", [128, 128])
    b1_rep = par("b1_rep", [128, 128])
    b2_rep = par("b2_rep", [128, 128])
    ident = par("ident", [128, 128])
    ridx_p = par("ridx", list(plan["ridx"][0].shape), I16)
    famp = {}
    for nm, fam in (("h", fam_h), ("t", fam_t), ("e", fam_e)):
        famp[nm] = dict(
            idxA=par(f"{nm}_idxA", list(fam["idxA"][0].shape), I16),
            idxB=par(f"{nm}_idxB", list(fam["idxB"][0].shape), I16),
            mask=par(f"{nm}_mask", list(fam["mask"][0].shape)),
        )
    out_ext = par("out", [NPAD, 128], out=True)

    proj_own = nc.dram_tensor("proj_own", [NPAD, 128], F32)
    d_attr = nc.dram_tensor("d_attr", [NTOT, 128], F32, addr_space="Shared")
    d_rel = nc.dram_tensor("d_rel", [NREL, 256], F32)
    triple = nc.dram_tensor("triple", [NPAD, 256], F32)
    h_own = nc.dram_tensor("h_own", [NPAD, 128], F32)
    d_h = nc.dram_tensor("d_h", [NTOT, 128], F32, addr_space="Shared")
    x2_own = nc.dram_tensor("x2_own", [NPAD, 128], F32)
    h2_own = nc.dram_tensor("h2_own", [NPAD, 128], F32)
    d_h2 = nc.dram_tensor("d_h2", [NTOT, 128], F32, addr_space="Shared")

    cmax = {
        "hA": max(fam_h["cA"]), "hB": max(fam_h["cB"]),
        "tA": max(fam_t["cA"]), "tB": max(fam_t["cB"]),
        "eA": max(fam_e["cA"]), "eB": max(fam_e["cB"]),
    }
    cmb_max = max(cmax["hA"] + cmax["hB"], cmax["tA"] + cmax["tB"],
                  cmax["eA"] + cmax["eB"])
    wcols = max(cmax.values()) * 128

    st = ExitStack()

    def sb(name, shape, dt=F32):
        return st.enter_context(nc.sbuf_tensor(name, list(shape), dt))

    def psum(name, shape):
        return st.enter_context(nc.psum_tensor(name, list(shape), F32))

    s_fembwt = sb("s_fembwt", [128, 128])
    s_femb_b = sb("s_femb_b", [128, 128])
    s_waug1 = sb("s_waug1", [128, 260])
    s_waug2 = sb("s_waug2", [128, 130])
    s_asrc = [sb("s_asrc1", [128, 128]), sb("s_asrc2", [128, 128])]
    s_bias = [sb("s_b1", [128, 128]), sb("s_b2", [128, 128])]
    s_ident = sb("s_ident", [128, 128])
    s_ridx = sb("s_ridx", [128, 8 * NTILE], I16)
    s_ad = [sb("s_ad1", [128, NTILE]), sb("s_ad2", [128, NTILE])]
    s_at = [sb(f"s_at{i}", [128, 128]) for i in range(2)]
    s_proj = [sb(f"s_proj{i}", [128, 128]) for i in range(2)]
    s_rel = [sb(f"s_rel{i}", [128, 256]) for i in range(2)]
    gbuf = {k: [sb(f"s_g{k}{i}", [128, cmax[k] * 128]) for i in range(2)]
            for k in cmax}
    ibuf = {k: [sb(f"s_i{k}{i}", [128, 8 * cmax[k]], I16) for i in range(2)]
            for k in cmax}
    mbuf = {k: [sb(f"s_m{k}{i}", [128, cmax[k + "A"] + cmax[k + "B"]])
                for i in range(2)] for k in ("h", "t", "e")}
    s_w1 = sb("s_w1", [128, wcols])
    s_sc = sb("s_sc", [128, cmb_max])
    s_ex = sb("s_ex", [128, cmb_max])
    s_red = sb("s_red", [128, 4])
    s_acc = sb("s_acc", [128, 128])
    s_acc2 = sb("s_acc2", [128, 128])
    s_emb = [sb("s_embh", [128, 128]), sb("s_embt", [128, 128])]
    s_x = [sb(f"s_x{i}", [128, 256]) for i in range(2)]
    s_xT = [sb(f"s_xT{i}", [128, 256]) for i in range(2)]
    s_h = [sb(f"s_h{i}", [128, 128]) for i in range(2)]
    s_o = [sb(f"s_o{i}", [128, 128]) for i in range(2)]
    s_z = sb("s_z", [128, 128])
    p_mm = [psum(f"p_mm{i}", [128, 130]) for i in range(2)]
    p_tr = [psum(f"p_tr{i}", [128, 128]) for i in range(2)]

    # ---------------- scheduling framework
    # DMA semaphores are split by purpose and tile parity so that every
    # wait covers the complete already-issued increment set on its sem
    # (race-detector-clean); compute sems (pe/act/dve/cc) update in issue
    # order and use plain cumulative counts.
    ENGS = ("gpsimd", "sync", "vector", "scalar", "tensor")
    SEMS = ("w", "p0a", "p0b", "ixa", "ixb", "gta", "gtb",
            "twa", "twb", "xa", "xb", "hwa", "hwb", "owa", "owb",
            "pe", "act", "dve", "cc", "gp", "msa", "msb", "pad")
    regs = {}
    ops = {e: [] for e in ENGS}
    cnt = {s: 0 for s in SEMS}
    last_wait = {e: {} for e in ENGS}

    def add(eng, emit, waits=(), inc=None):
        # same-engine pipelining can reorder element accesses: serialize
        # vector/scalar streams against themselves via their own sem.
        if eng == "vector":
            waits = list(waits) + [("dve", cnt["dve"])]
        elif eng == "scalar":
            waits = list(waits) + [("act", cnt["act"])]
        w = []
        for s_name, val in waits:
            if val <= 0 or last_wait[eng].get(s_name, -1) >= val:
                continue
            last_wait[eng][s_name] = val
            w.append((s_name, val))
        ops[eng].append((emit, tuple(w), inc))
        if inc:
            cnt[inc[0]] += inc[1]
        return dict(cnt)

    def pt(base, t):
        return base + ("a" if t % 2 == 0 else "b")

    def view_cf(buf_ap, c):      # [128, c*128] -> [128, c, 128]
        return buf_ap.rearrange("p (c f) -> p c f", f=128)

    def rep_mid(vec_ap, c):      # [128, 128] -> [128, c, 128] (0-step mid)
        return vec_ap.unsqueeze(1).broadcast_to([vec_ap.shape[0], c, 128])

    def exp_inner(sc_ap, c):     # [128, c] -> [128, c, 128] (0-step inner)
        return sc_ap.unsqueeze(2).broadcast_to([sc_ap.shape[0], c, 128])

    def jview(buf_ap, c):        # [128, c*128] -> [128, 128, c] (j innermost)
        return buf_ap.rearrange("p (c f) -> p c f", f=128).transpose([0, 2, 1])

    # ---------------- phase W: constants
    for dst, srcp in ((s_fembwt, femb_wt), (s_femb_b, femb_b_rep),
                      (s_waug2, waug2), (s_asrc[0], asrc1_rep),
                      (s_asrc[1], asrc2_rep), (s_bias[0], b1_rep),
                      (s_bias[1], b2_rep), (s_ident, ident), (s_ridx, ridx_p)):
        add("sync", lambda s, d=dst, so=srcp: s.dma_start(
            out=d[:, :], in_=so[:, :]), inc=("w", 16))
    add("sync", lambda s: s.dma_start(out=s_waug1[:, 0:130],
                                      in_=waug1[0:128, :]), inc=("w", 16))
    add("sync", lambda s: s.dma_start(out=s_waug1[:, 130:260],
                                      in_=waug1[128:256, :]), inc=("w", 16))
    W = cnt["w"]

    # ---------------- phase 0: table projections
    def proj_rows(src_cols, n_rows, out_dst, marks):
        ntl = (n_rows + 127) // 128
        for t in range(ntl):
            b = t % 2
            m = min(128, n_rows - t * 128)
            c0 = t * 128
            snap = add("sync", lambda s, b=b, c0=c0, m=m, sc=src_cols:
                       s.dma_start(out=s_at[b][:, 0:m],
                                   in_=sc[:, c0 : c0 + m]),
                       waits=[("pe", marks.get(("pe", b), 0))],
                       inc=(pt("p0", t), 16))
            snap = add("tensor", lambda te, b=b, m=m: te.matmul(
                p_tr[b][0:m, :], s_at[b][:, 0:m], s_fembwt[:, :],
                start=True, stop=True),
                waits=[(pt("p0", t), snap[pt("p0", t)]), ("w", W),
                       ("dve", marks.get(("dve", b), 0))],
                inc=("pe", 1))
            marks[("pe", b)] = snap["pe"]
            ms = pt("ms", t)
            snap = add("vector", lambda v, b=b, m=m: v.tensor_tensor(
                out=s_proj[b][0:m, :], in0=p_tr[b][0:m, :],
                in1=s_femb_b[0:m, :], op=mb.AluOpType.add),
                waits=[("pe", snap["pe"]), ("w", W),
                       (ms, marks.get(("ms", b), 0))],
                inc=("dve", 1))
            marks[("dve", b)] = snap["dve"]
            snap = add("gpsimd", lambda g, b=b, c0=c0, m=m, od=out_dst:
                       g.dma_start(out=od(c0, m), in_=s_proj[b][0:m, :]),
                       waits=[("dve", snap["dve"])], inc=(ms, 16))
            marks[("ms", b)] = snap[ms]
        return marks

    marks = proj_rows(attr_tT, SHARD,
                      lambda c0, m: proj_own[c0 : c0 + m, :], {})
    snap = add("gpsimd", lambda g: g.memset(s_z[:, :], 0.0), inc=("gp", 1))
    add("gpsimd", lambda g: g.dma_start(
        out=proj_own[SHARD:NPAD, :], in_=s_z[0 : NPAD - SHARD, :]),
        waits=[("gp", snap["gp"])], inc=("pad", 16))
    marks = proj_rows(rel_tT, NREL,
                      lambda c0, m: d_rel[c0 : c0 + m, 0:128], marks)
    add("gpsimd", lambda g: g.dma_start(out=d_rel[:, 128:256],
                                        in_=rel_rows[:, :]), inc=("pad", 16))
    MSA, MSB, GP = cnt["msa"], cnt["msb"], cnt["pad"]

    snap = add("gpsimd", lambda g: g.collective_compute(
        "AllGather", mb.AluOpType.bypass,
        replica_groups=[list(range(NCORE))],
        ins=[proj_own[:, :]], outs=[d_attr[:, :]]),
        waits=[("msa", MSA), ("msb", MSB), ("pad", GP)], inc=("cc", 1))
    cc_attr = snap["cc"]

    # ---------------- families: offsets
    offs = {"h": [_fam_off(fam_h, t) for t in range(NTILE + 1)],
            "t": [_fam_off(fam_t, t) for t in range(NTILE + 1)],
            "e": [_fam_off(fam_e, t) for t in range(NTILE + 1)]}

    def issue_idx(nm, fam, t, b, reuse_dve, reuse_gt):
        oA, oB, oM = offs[nm][t]
        cA, cB = fam["cA"][t], fam["cB"][t]
        pars = famp[nm]
        iA, iB = ibuf[nm + "A"][b], ibuf[nm + "B"][b]
        mB = mbuf[nm][b]
        ix, gt = pt("ix", t), pt("gt", t)
        add("sync", lambda s, iA=iA, oA=oA, cA=cA, pars=pars: s.dma_start(
            out=iA[:, 0 : 8 * cA], in_=pars["idxA"][:, oA : oA + 8 * cA]),
            waits=[(gt, reuse_gt), ("w", W)], inc=(ix, 16))
        add("sync", lambda s, iB=iB, oB=oB, cB=cB, pars=pars: s.dma_start(
            out=iB[:, 0 : 8 * cB], in_=pars["idxB"][:, oB : oB + 8 * cB]),
            inc=(ix, 16))
        snap = add("sync", lambda s, mB=mB, oM=oM, cc2=cA + cB, pars=pars:
                   s.dma_start(out=mB[:, 0 : cc2],
                               in_=pars["mask"][:, oM : oM + cc2]),
                   waits=[("dve", reuse_dve)], inc=(ix, 16))
        return snap

    def issue_gat(nm, fam, t, b, tabA, tabB, ix_snap, reuse_dve, extra_gw=()):
        cA, cB = fam["cA"][t], fam["cB"][t]
        bA, bB = gbuf[nm + "A"][b], gbuf[nm + "B"][b]
        iA, iB = ibuf[nm + "A"][b], ibuf[nm + "B"][b]
        ix, gt = pt("ix", t), pt("gt", t)
        gw = ([(ix, ix_snap[ix]), ("dve", reuse_dve)] + list(extra_gw))

        GCHUNK = 16  # ucode packet/ring limits: stay <= 2048 idx per gather

        def _gather(g, buf, ib, c0, c1, tab):
            g.reg_mov(regs["g"], 128 * (c1 - c0))
            return g.dma_gather(
                out_ap=view_cf(buf[:, c0 * 128 : c1 * 128], c1 - c0),
                in_ap=tab, idxs_ap=ib[:, 8 * c0 : 8 * c1],
                num_idxs=128 * (c1 - c0), num_idxs_reg=regs["g"],
                elem_size=128, single_packet=False)

        snap = None
        for buf, ib, cX, tab in ((bA, iA, cA, tabA), (bB, iB, cB, tabB)):
            for c0 in range(0, cX, GCHUNK):
                c1 = min(c0 + GCHUNK, cX)
                snap = add("gpsimd",
                           lambda g, buf=buf, ib=ib, c0=c0, c1=c1, tab=tab:
                           _gather(g, buf, ib, c0, c1, tab),
                           waits=gw, inc=(gt, 16))
        return snap, cA, cB

    def attention(nm, cA, cB, b, query_ap_fn, first_waits, extra_ap=None,
                  lrelu=False):
        c = cA + cB
        bufs = (gbuf[nm + "A"][b], gbuf[nm + "B"][b])
        mask = mbuf[nm][b]
        for i, (cX, buf, o0) in enumerate(((cA, bufs[0], 0),
                                           (cB, bufs[1], cA))):
            q_ap = query_ap_fn(cX)
            add("vector", lambda v, cX=cX, buf=buf, q=q_ap: v.tensor_tensor(
                out=view_cf(s_w1[:, 0 : cX * 128], cX),
                in0=view_cf(buf[:, 0 : cX * 128], cX), in1=q,
                op=mb.AluOpType.mult),
                waits=first_waits if i == 0 else (), inc=("dve", 1))
            add("vector", lambda v, cX=cX, o0=o0: v.tensor_reduce(
                out=s_sc[:, o0 : o0 + cX],
                in_=view_cf(s_w1[:, 0 : cX * 128], cX),
                axis=mb.AxisListType.X, op=mb.AluOpType.add), inc=("dve", 1))
        if extra_ap is not None:
            add("vector", lambda v, e=extra_ap, c=c: v.tensor_scalar_add(
                s_sc[:, 0:c], s_sc[:, 0:c], e), inc=("dve", 1))
        add("vector", lambda v, c=c, mask=mask: v.tensor_tensor(
            out=s_sc[:, 0:c], in0=s_sc[:, 0:c], in1=mask[:, 0:c],
            op=mb.AluOpType.add), inc=("dve", 1))
        if lrelu:
            add("vector", lambda v, c=c: v.tensor_scalar_mul(
                s_ex[:, 0:c], s_sc[:, 0:c], NEG_SLOPE), inc=("dve", 1))
            add("vector", lambda v, c=c: v.tensor_tensor(
                out=s_sc[:, 0:c], in0=s_sc[:, 0:c], in1=s_ex[:, 0:c],
                op=mb.AluOpType.max), inc=("dve", 1))
        snap = add("vector", lambda v, c=c: v.tensor_reduce(
            out=s_red[:, 0:1], in_=s_sc[:, 0:c], axis=mb.AxisListType.X,
            op=mb.AluOpType.max, negate=True), inc=("dve", 1))
        snap = add("scalar", lambda sc, c=c: sc.activation(
            out=s_ex[:, 0:c], in_=s_sc[:, 0:c],
            func=mb.ActivationFunctionType.Exp,
            bias=s_red[:, 0:1], accum_out=s_red[:, 1:2]),
            waits=[("dve", snap["dve"])], inc=("act", 1))
        snap = add("vector", lambda v: v.reciprocal(s_red[:, 2:3],
                                                    s_red[:, 1:2]),
                   waits=[("act", snap["act"])], inc=("dve", 1))
        for i, (cX, buf, o0) in enumerate(((cA, bufs[0], 0),
                                           (cB, bufs[1], cA))):
            acc = s_acc if i == 0 else s_acc2
            add("vector", lambda v, cX=cX, buf=buf, o0=o0: v.tensor_tensor(
                out=view_cf(s_w1[:, 0 : cX * 128], cX),
                in0=view_cf(buf[:, 0 : cX * 128], cX),
                in1=exp_inner(s_ex[:, o0 : o0 + cX], cX),
                op=mb.AluOpType.mult), inc=("dve", 1))
            add("vector", lambda v, cX=cX, acc=acc: v.tensor_reduce(
                out=acc[:, :], in_=jview(s_w1[:, 0 : cX * 128], cX),
                axis=mb.AxisListType.X, op=mb.AluOpType.add), inc=("dve", 1))
        snap = add("vector", lambda v: v.tensor_tensor(
            out=s_acc[:, :], in0=s_acc[:, :], in1=s_acc2[:, :],
            op=mb.AluOpType.add), inc=("dve", 1))
        return snap

    import os as _os
    _STOP = int(_os.environ.get("BUILD_STOP", "9"))
    if _STOP < 1:
        NT1 = 0
    else:
        NT1 = NTILE
    # ---------------- phase 1: entity embedding
    emb_dve_done, emb_gt_done, emb_tw = {}, {}, {}
    for t in range(NT1):
        b = t % 2
        gt, tw = pt("gt", t), pt("tw", t)
        reuse_d = emb_dve_done.get(t - 2, 0)
        reuse_gt = emb_gt_done.get(t - 2, 0)

        def _relgather(g, t, b):
            g.reg_mov(regs["g"], 128)
            return g.dma_gather(
                out_ap=s_rel[b][:, :].unsqueeze(1),
                in_ap=d_rel[:, :], idxs_ap=s_ridx[:, 8 * t : 8 * t + 8],
                num_idxs=128, num_idxs_reg=regs["g"], elem_size=256)
        snap = add("gpsimd", lambda g, t=t, b=b: _relgather(g, t, b),
                   waits=[("cc", cc_attr), ("w", W), ("msa", MSA),
                          ("msb", MSB), ("pad", GP), ("dve", reuse_d),
                          (tw, emb_tw.get(t - 2, 0))],
                   inc=(gt, 16))
        issue_idx("h", fam_h, t, b, reuse_d, reuse_gt)
        ix_snap = issue_idx("t", fam_t, t, b, reuse_d, reuse_gt)
        snap, cAh, cBh = issue_gat(
            "h", fam_h, t, b, d_attr[0:SPLIT, :], d_attr[SPLIT:NTOT, :],
            ix_snap, reuse_d, [("cc", cc_attr)])
        snap, cAt, cBt = issue_gat(
            "t", fam_t, t, b, d_attr[0:SPLIT, :], d_attr[SPLIT:NTOT, :],
            ix_snap, reuse_d)
        emb_gt_done[t] = snap[gt]
        gw = [(gt, snap[gt])]
        rp_fn = lambda cX, b=b: rep_mid(s_rel[b][:, 0:128], cX)
        attention("h", cAh, cBh, b, rp_fn, gw)
        add("vector", lambda v: v.tensor_scalar_mul(
            s_emb[0][:, :], s_acc[:, :], s_red[:, 2:3]), inc=("dve", 1))
        attention("t", cAt, cBt, b, rp_fn, ())
        add("vector", lambda v: v.tensor_scalar_mul(
            s_emb[1][:, :], s_acc[:, :], s_red[:, 2:3]), inc=("dve", 1))
        snap = add("vector", lambda v, b=b: v.tensor_tensor(
            out=s_o[b][:, :], in0=s_emb[0][:, :], in1=s_emb[1][:, :],
            op=mb.AluOpType.add),
            waits=[(pt("ow", t), 0)], inc=("dve", 1))
        emb_dve_done[t] = snap["dve"]
        add("gpsimd", lambda g, t=t, b=b: g.dma_start(
            out=triple[128 * t : 128 * (t + 1), 0:128], in_=s_o[b][:, :]),
            waits=[("dve", snap["dve"])], inc=(tw, 16))
        snap = add("gpsimd", lambda g, t=t, b=b: g.dma_start(
            out=triple[128 * t : 128 * (t + 1), 128:256],
            in_=s_rel[b][:, 128:256]), inc=(tw, 16))
        emb_tw[t] = snap[tw]
    TWA, TWB = cnt["twa"], cnt["twb"]
    if _STOP < 2:
        NTILE_MM = 0
    else:
        NTILE_MM = NTILE

    # ---------------- GAT dense matmuls (phase-barriered on inputs)
    def gat_matmul(layer, x_src, nchunks, h_dst, in_waits):
        mm_act, h_hw = {}, {}
        waug = s_waug1 if layer == 1 else s_waug2
        for t in range(NTILE_MM):
            b = t % 2
            x, hw = pt("x", t), pt("hw", t)
            snap = add("sync", lambda s, t=t, b=b, nk=nchunks: s.dma_start(
                out=s_x[b][:, 0 : 128 * nk], in_=x_src(t)),
                waits=list(in_waits) + [("act", mm_act.get(t - 2, 0))],
                inc=(x, 16))
            sd = snap[x]
            a_snap = 0
            for k in range(nchunks):
                snap = add("tensor", lambda te, b=b, k=k: te.transpose(
                    out=p_tr[b][:, :], in_=s_x[b][:, 128 * k : 128 * (k + 1)],
                    identity=s_ident[:, :]),
                    waits=[(x, sd), ("act", a_snap), ("w", W)], inc=("pe", 1))
                snap = add("scalar", lambda sc, b=b, k=k: sc.activation(
                    out=s_xT[b][:, 128 * k : 128 * (k + 1)],
                    in_=p_tr[b][:, :], func=mb.ActivationFunctionType.Copy),
                    waits=[("pe", snap["pe"])], inc=("act", 1))
                a_snap = snap["act"]
            for k in range(nchunks):
                snap = add("tensor", lambda te, b=b, k=k, waug=waug,
                           nk=nchunks: te.matmul(
                    p_mm[b][:, :], s_xT[b][:, 128 * k : 128 * (k + 1)],
                    waug[:, 130 * k : 130 * (k + 1)],
                    start=(k == 0), stop=(k == nk - 1)),
                    waits=[("act", a_snap)], inc=("pe", 1))
            snap = add("scalar", lambda sc, b=b: sc.activation(
                out=s_h[b][:, :], in_=p_mm[b][:, 0:128],
                func=mb.ActivationFunctionType.Copy),
                waits=[("pe", snap["pe"]), (hw, h_hw.get(t - 2, 0))],
                inc=("act", 1))
            snap = add("scalar", lambda sc, b=b, t=t, lay=layer: sc.activation(
                out=s_ad[lay - 1][:, t : t + 1], in_=p_mm[b][:, 129:130],
                func=mb.ActivationFunctionType.Copy), inc=("act", 1))
            mm_act[t] = snap["act"]
            snap = add("gpsimd", lambda g, t=t, b=b, hd=h_dst: g.dma_start(
                out=hd[128 * t : 128 * (t + 1), :], in_=s_h[b][:, :]),
                waits=[("act", snap["act"])], inc=(hw, 16))
            h_hw[t] = snap[hw]
        return dict(cnt)

    mm1 = gat_matmul(1, lambda t: triple[128 * t : 128 * (t + 1), :], 2,
                     h_own, [("twa", TWA), ("twb", TWB)])
    if _STOP >= 3:
        snap = add("gpsimd", lambda g: g.collective_compute(
            "AllGather", mb.AluOpType.bypass,
            replica_groups=[list(range(NCORE))],
            ins=[h_own[:, :]], outs=[d_h[:, :]]),
            waits=[("hwa", mm1["hwa"]), ("hwb", mm1["hwb"])], inc=("cc", 1))
        cc_h1 = snap["cc"]
    else:
        cc_h1 = 0

    # ---------------- edge phases
    def edge_phase(layer, d_tab, out_dst, cc_need):
        ed_done, ed_gt, ed_ow = {}, {}, {}
        bias = s_bias[layer - 1]
        asr = s_asrc[layer - 1]
        ad_col = s_ad[layer - 1]
        for g_i in range(NTILE if _STOP >= 4 else 0):
            b = g_i % 2
            gt, ow = pt("gt", g_i), pt("ow", g_i)
            reuse_d = ed_done.get(g_i - 2, 0)
            reuse_gt = ed_gt.get(g_i - 2, 0)
            ix_snap = issue_idx("e", fam_e, g_i, b, reuse_d, reuse_gt)
            snap, cA, cB = issue_gat(
                "e", fam_e, g_i, b, d_tab[0:SPLIT, :], d_tab[SPLIT:NTOT, :],
                ix_snap, reuse_d, [("cc", cc_need)])
            ed_gt[g_i] = snap[gt]
            gw = [(gt, snap[gt])]
            q_fn = lambda cX, asr=asr: rep_mid(asr[:, 0:128], cX)
            attention("e", cA, cB, b, q_fn, gw,
                      extra_ap=ad_col[:, g_i : g_i + 1], lrelu=True)
            snap = add("vector", lambda v, b=b: v.tensor_scalar_mul(
                s_o[b][:, :], s_acc[:, :], s_red[:, 2:3]),
                waits=[(ow, ed_ow.get(g_i - 2, 0))], inc=("dve", 1))
            snap = add("vector", lambda v, b=b, bias=bias: v.tensor_tensor(
                out=s_o[b][:, :], in0=s_o[b][:, :], in1=bias[:, :],
                op=mb.AluOpType.add), inc=("dve", 1))
            ed_done[g_i] = snap["dve"]
            snap = add("gpsimd", lambda g, g_i=g_i, b=b, od=out_dst:
                       g.dma_start(
                           out=od[128 * g_i : 128 * (g_i + 1), :],
                           in_=s_o[b][:, :]),
                       waits=[("dve", snap["dve"])], inc=(ow, 16))
            ed_ow[g_i] = snap[ow]
        return dict(cnt)

    e1 = edge_phase(1, d_h, x2_own, cc_h1)
    if _STOP >= 5:
        mm2 = gat_matmul(2, lambda t: x2_own[128 * t : 128 * (t + 1), :], 1,
                         h2_own, [("owa", e1["owa"]), ("owb", e1["owb"])])
        snap = add("gpsimd", lambda g: g.collective_compute(
            "AllGather", mb.AluOpType.bypass,
            replica_groups=[list(range(NCORE))],
            ins=[h2_own[:, :]], outs=[d_h2[:, :]]),
            waits=[("hwa", mm2["hwa"]), ("hwb", mm2["hwb"])], inc=("cc", 1))
        cc_h2 = snap["cc"]
        if _STOP >= 6:
            edge_phase(2, d_h2, out_ext, cc_h2)

    if _STOP < 9:
        snap0 = add("gpsimd", lambda g: g.dma_start(
            out=out_ext[0:128, :], in_=s_z[:, :]), inc=("pad", 16))
    final = dict(cnt)
    import os
    if os.environ.get("BASS_PRINT_SEMS"):
        print("FINAL SEM COUNTS:", final)

    # ---------------- emit
    with ExitStack() as es:
        block = es.enter_context(nc.Block())
        sems = {s_name: es.enter_context(nc.semaphore(f"sem_{s_name}"))
                for s_name in SEMS}

        def make_body(eng_name):
            def body(eng):
                if eng_name == "gpsimd":
                    regs["g"] = es.enter_context(eng.register("gnum"))
                for emit, waits, inc in ops[eng_name]:
                    for s_name, val in waits:
                        eng.wait_ge(sems[s_name], val)
                    inst = emit(eng)
                    if inc is not None and inst is not None:
                        inst.then_inc(sems[inc[0]], inc[1])
                if eng_name == "gpsimd":
                    for s_name in SEMS:
                        if s_name != "cc" and final[s_name] > 0:
                            eng.wait_ge(sems[s_name], final[s_name])
            return body

        block.gpsimd(make_body("gpsimd"))
        block.sync(make_body("sync"))
        block.vector(make_body("vector"))
        block.scalar(make_body("scalar"))
        block.tensor(make_body("tensor"))

    nc.compile()
    st.close()
    return nc


# ---------------------------------------------------------------- kernel() --

_CACHE = {}


def _prepare(inputs):
    plan = make_plan(inputs["h_attributes"], inputs["t_attributes"],
                     inputs["r_idx"], inputs["edge_index"])
    weights = make_weights(
        inputs["attr_table"], inputs["rel_table"], inputs["femb_w"],
        inputs["femb_b"], inputs["gat1_w"], inputs["gat1_asrc"],
        inputs["gat1_adst"], inputs["gat1_b"], inputs["gat2_w"],
        inputs["gat2_asrc"], inputs["gat2_adst"], inputs["gat2_b"])
    in_maps = []
    for c in range(NCORE):
        m = dict(
            attr_tT=np.ascontiguousarray(
                np.roll(weights["attr_tT"], -c * SHARD, axis=1)),
            rel_tT=weights["rel_tT"], rel_rows=weights["rel_rows"],
            femb_wt=weights["femb_wt"], femb_b_rep=weights["femb_b_rep"],
            waug1=weights["waug1"], waug2=weights["waug2"],
            asrc1_rep=weights["asrc1_rep"], asrc2_rep=weights["asrc2_rep"],
            b1_rep=weights["b1_rep"], b2_rep=weights["b2_rep"],
            ident=weights["ident"], ridx=plan["ridx"][c],
        )
        for nm in ("h", "t", "e"):
            fam = plan[f"fam_{nm}"]
            m[f"{nm}_idxA"] = fam["idxA"][c]
            m[f"{nm}_idxB"] = fam["idxB"][c]
            m[f"{nm}_mask"] = fam["mask"][c]
        in_maps.append(m)
    return plan, weights, in_maps


LAST_EXEC_NS = None


def kernel(**inputs):
    global LAST_EXEC_NS
    import os
    plan, weights, in_maps = _prepare(inputs)
    nc = build_bass(plan)
    from concourse.bass_utils import run_bass_kernel_spmd
    trace = bool(os.environ.get("BASS_KTRACE"))
    res = run_bass_kernel_spmd(nc, in_maps, list(range(NCORE)), trace=trace)
    if res.exec_time_ns is not None:
        LAST_EXEC_NS = res.exec_time_ns
    outs = np.stack([np.asarray(res.results[c]["out"]) for c in range(NCORE)])
    return np.ascontiguousarray(
        outs[plan["core_of"], plan["local_of"]].astype(np.float32))


def bench(n_iter=5, inputs=None):
    """Time repeated NEFF executions (inputs staged once; outputs donated
    fresh each iter). Returns (best_s, all_s)."""
    import time
    import jax
    import jax.numpy as jnp
    from jax.sharding import Mesh, PartitionSpec, NamedSharding
    plan, weights, in_maps = _prepare(inputs)
    nc = build_bass(plan)
    from concourse import bass2jax

    # replicate run_bass_via_pjrt, but keep the compiled callable
    import concourse.mybir as mybir
    bass2jax.install_neuronx_cc_hook()
    partition_name = (nc.partition_id_tensor.name
                      if nc.partition_id_tensor else None)
    in_names, out_names, out_avals, zero_outs = [], [], [], []
    for alloc in nc.m.functions[0].allocations:
        if not isinstance(alloc, mybir.MemoryLocationSet):
            continue
        name = alloc.memorylocations[0].name
        if alloc.kind == "ExternalInput":
            if name != partition_name:
                in_names.append(name)
        elif alloc.kind == "ExternalOutput":
            shape = tuple(alloc.tensor_shape)
            dtype = mybir.dt.np(alloc.dtype)
            out_names.append(name)
            out_avals.append(jax.core.ShapedArray(shape, dtype))
            zero_outs.append(np.zeros(shape, dtype))
    n_params = len(in_names)
    n_outs = len(out_avals)
    in_names.extend(out_names)
    if partition_name is not None:
        in_names.append(partition_name)
    donate = tuple(range(n_params, n_params + n_outs))

    def _body(*args):
        operands = list(args)
        if partition_name is not None:
            operands.append(bass2jax.partition_id_tensor())
        return tuple(bass2jax._bass_exec_p.bind(
            *operands, out_avals=tuple(out_avals), in_names=tuple(in_names),
            out_names=tuple(out_names), lowering_input_output_aliases=(),
            sim_require_finite=True, sim_require_nnan=True, nc=nc))

    from jax.experimental.shard_map import shard_map
    devices = jax.devices()[:NCORE]
    mesh = Mesh(np.asarray(devices), ("core",))
    in_specs = (PartitionSpec("core"),) * (n_params + n_outs)
    out_specs = (PartitionSpec("core"),) * len(out_names)
    fn = jax.jit(shard_map(_body, mesh=mesh, in_specs=in_specs,
                           out_specs=out_specs, check_rep=False),
                 donate_argnums=donate, keep_unused=True)
    if nc.dbg_addr is not None:
        in_maps = [{**m, nc.dbg_addr.name: np.zeros((1, 2), np.uint32)}
                   for m in in_maps]
    per_core = [[np.asarray(m[k]) for k in in_names[:n_params]]
                for m in in_maps]
    sh = NamedSharding(mesh, PartitionSpec("core"))
    concat_in = [jax.device_put(
        np.concatenate([per_core[c][i] for c in range(NCORE)], axis=0), sh)
        for i in range(n_params)]
    zglobal = [np.zeros((NCORE * z.shape[0], *z.shape[1:]), z.dtype)
               for z in zero_outs]
    times = []
    for it in range(n_iter):
        zs = [jax.device_put(z, sh) for z in zglobal]
        for z in zs:
            z.block_until_ready()
        t0 = time.perf_counter()
        outs = fn(*concat_in, *zs)
        for o in outs:
            o.block_until_ready()
        times.append(time.perf_counter() - t0)
    return min(times), times



# revision 2
# speedup vs baseline: 1.2080x; 1.2080x over previous
"""Distributed Trainium2 kernel for AttributeHypergraphModel (2x GATConv over
triples with attribute-attention entity embeddings).

Strategy (8 NeuronCores, SPMD):
  - nodes are relabeled on the host: sorted by (in-degree, A-side edge count)
    and dealt round-robin to cores, so every core's tile t has near-identical
    padded shapes (required: one SPMD graph) and padded gather groups waste
    little traffic.
  - attr/rel tables are projected once on device (matmul); the projected attr
    table is sharded + AllGathered. Entity-embedding attention and both GAT
    layers then run on dma_gather'ed rows (A/B split tables keep gather
    indices under the signed-int16 ucode limit; -1e30 mask planes neutralize
    padding slots).
  - each GAT layer: dense matmul with folded alpha_dst column, AllGather of
    node features, dst-partitioned softmax + weighted sum per 128-dst group.
All index/mask planes are precomputed host-side; outputs are un-permuted on
the host.
"""

import sys

sys.path.insert(0, "/opt/trn_rl_repo")

import numpy as np

NCORE = 8
N = 50000
A = 16
NREL = 500
DE = 128
NPAD = 6272  # 49 tiles of 128 local slots per core
NTILE = NPAD // 128
NTOT = NPAD * NCORE  # 50176 global slots
SHARD = N // NCORE  # 6250 real rows per core (attr table + nodes)
SPLIT = 32768
NEGB = np.float32(-1.0e30)
NEG_SLOPE = 0.2


# ---------------------------------------------------------------- planning --


def _pack_idx(plane):
    """[128, c] int plane (slot p gets column j at gather position j*128+p)
    -> int16 SBUF index layout [128, 8*c] (16-row pattern replicated x8)."""
    p128, c = plane.shape
    assert p128 == 128
    assert plane.min(initial=0) >= 0 and plane.max(initial=0) < 32768
    vals = plane.T.reshape(-1)  # logical gather order
    cols = vals.size // 16
    arr = vals.reshape(cols, 16).T  # arr[i%16, i//16] = vals[i]
    return np.ascontiguousarray(np.tile(arr, (8, 1)).astype(np.int16))


def _column_planes(padded, k_a, total, c_a, c_b, split):
    """Split per-row id lists (A-first order in `padded`) into A/B column
    planes plus additive mask biases (-1e30 on padding)."""
    colA = np.arange(c_a)[None, :]
    mA = colA < k_a[:, None]
    pA = np.where(mA, padded[:, :c_a], 0).astype(np.int64)
    bA = np.where(mA, np.float32(0), NEGB).astype(np.float32)
    colB = np.arange(c_b)[None, :]
    mB = colB < (total - k_a)[:, None]
    gidx = np.minimum(k_a[:, None] + colB, padded.shape[1] - 1)
    pB = np.where(mB, np.take_along_axis(padded, gidx, axis=1) - split, 0)
    pB = pB.astype(np.int64)
    bB = np.where(mB, np.float32(0), NEGB).astype(np.float32)
    return pA, bA, pB, bB


def _build_family(ordered, kA, total, cA, cB, split):
    """ordered: [NCORE*NPAD, W] id lists (A ids first); returns per-tile
    cA/cB and per-core concatenated idx/mask planes."""
    nrow = ordered.shape[0]
    per_core = nrow // NCORE
    ntile = per_core // 128
    idx_a = [[] for _ in range(NCORE)]
    idx_b = [[] for _ in range(NCORE)]
    masks = [[] for _ in range(NCORE)]
    for c in range(NCORE):
        for t in range(ntile):
            r0 = c * per_core + t * 128
            pA, bA, pB, bB = _column_planes(
                ordered[r0 : r0 + 128], kA[r0 : r0 + 128], total[r0 : r0 + 128],
                int(cA[t]), int(cB[t]), split,
            )
            idx_a[c].append(_pack_idx(pA))
            idx_b[c].append(_pack_idx(pB))
            masks[c].append(np.concatenate([bA, bB], axis=1))
    return dict(
        cA=[int(x) for x in cA],
        cB=[int(x) for x in cB],
        idxA=[np.ascontiguousarray(np.concatenate(v, axis=1)) for v in idx_a],
        idxB=[np.ascontiguousarray(np.concatenate(v, axis=1)) for v in idx_b],
        mask=[np.ascontiguousarray(np.concatenate(v, axis=1)) for v in masks],
    )


def _family_from_lists(ids, valid, split):
    """ids: [NCORE*NPAD, A] raw ids (already in table-slot space), valid rows
    marked; builds A-first ordering then the family planes."""
    ids = np.where(ids < 0, 0, ids)
    isB = ids >= split
    perm = np.argsort(isB, axis=1, kind="stable")
    ordered = np.take_along_axis(ids, perm, axis=1)
    kA = (~isB).sum(axis=1).astype(np.int64)
    total = np.full(len(ids), ids.shape[1], np.int64)
    kA[~valid] = 0
    total[~valid] = 0
    ordered = np.concatenate([ordered, np.zeros_like(ordered)], axis=1)
    kA3 = kA.reshape(NCORE, NTILE, 128)
    tot3 = total.reshape(NCORE, NTILE, 128)
    cA = np.maximum(kA3.max(axis=(0, 2)), 1)
    cB = np.maximum((tot3 - kA3).max(axis=(0, 2)), 1)
    return _build_family(ordered, kA, total, cA, cB, split)


def _remap_attr(ids):
    """raw attr id -> row in the padded AllGather'ed projection table."""
    return (ids // SHARD) * NPAD + (ids % SHARD)


def make_plan(h_attributes, t_attributes, r_idx, edge_index):
    h_attributes = np.asarray(h_attributes)
    t_attributes = np.asarray(t_attributes)
    r_idx = np.asarray(r_idx)
    edge_index = np.asarray(edge_index)

    src0 = np.concatenate([edge_index[0], np.arange(N, dtype=np.int64)])
    dst0 = np.concatenate([edge_index[1], np.arange(N, dtype=np.int64)])
    deg = np.bincount(dst0, minlength=N)

    def slots_from_order(order):
        rank = np.empty(N, np.int64)
        rank[order] = np.arange(N)
        core_of = rank % NCORE
        local_of = rank // NCORE
        return core_of * NPAD + local_of, core_of, local_of

    g0, _, _ = slots_from_order(np.argsort(deg, kind="stable"))
    kAe0 = np.bincount(dst0[g0[src0] < SPLIT], minlength=N)
    order = np.lexsort((kAe0, deg))
    gslot, core_of, local_of = slots_from_order(order)

    # ---- attr families (ids remapped into padded projection-table space)
    attrs_h = np.full((NCORE * NPAD, A), -1, np.int64)
    attrs_t = np.full((NCORE * NPAD, A), -1, np.int64)
    valid = np.zeros(NCORE * NPAD, bool)
    attrs_h[gslot] = _remap_attr(h_attributes)
    attrs_t[gslot] = _remap_attr(t_attributes)
    valid[gslot] = True
    fam_h = _family_from_lists(attrs_h, valid, SPLIT)
    fam_t = _family_from_lists(attrs_t, valid, SPLIT)

    # ---- r_idx gather planes
    r_slot = np.zeros(NCORE * NPAD, np.int64)
    r_slot[gslot] = r_idx
    r_slot = r_slot.reshape(NCORE, NPAD)
    ridx_planes = []
    for c in range(NCORE):
        cols = [_pack_idx(r_slot[c, t * 128 : (t + 1) * 128][:, None])
                for t in range(NTILE)]
        ridx_planes.append(np.ascontiguousarray(np.concatenate(cols, axis=1)))

    # ---- edge family (per-dst in-edge src slots, A-first)
    sg = gslot[src0]
    dg = gslot[dst0]
    order_e = np.lexsort(((sg >= SPLIT).astype(np.int64), dg))
    sg_s = sg[order_e]
    dg_s = dg[order_e]
    cnt = np.bincount(dg_s, minlength=NTOT)
    starts = np.concatenate([[0], np.cumsum(cnt)[:-1]])
    pos = np.arange(len(sg_s)) - starts[dg_s]
    maxdeg = int(cnt.max())
    padded_e = np.zeros((NTOT, maxdeg + 8), np.int64)
    padded_e[dg_s, pos] = sg_s
    kAe = np.bincount(dg_s[sg_s < SPLIT], minlength=NTOT).astype(np.int64)
    tot_e = cnt.astype(np.int64)
    kA3 = kAe.reshape(NCORE, NTILE, 128)
    tot3 = tot_e.reshape(NCORE, NTILE, 128)
    cAe = np.maximum(kA3.max(axis=(0, 2)), 1)
    cBe = np.maximum((tot3 - kA3).max(axis=(0, 2)), 1)
    need = int(cAe.max() + cBe.max())
    if padded_e.shape[1] < need:
        padded_e = np.concatenate(
            [padded_e, np.zeros((NTOT, need - padded_e.shape[1]), np.int64)],
            axis=1)
    fam_e = _build_family(padded_e, kAe, tot_e, cAe, cBe, SPLIT)

    return dict(core_of=core_of, local_of=local_of,
                fam_h=fam_h, fam_t=fam_t, fam_e=fam_e, ridx=ridx_planes)


def make_weights(attr_table, rel_table, femb_w, femb_b,
                 gat1_w, gat1_asrc, gat1_adst, gat1_b,
                 gat2_w, gat2_asrc, gat2_adst, gat2_b):
    f32 = np.float32
    w = {}
    w["attr_tT"] = np.ascontiguousarray(np.asarray(attr_table, f32).T)
    w["rel_tT"] = np.ascontiguousarray(np.asarray(rel_table, f32).T)
    w["rel_rows"] = np.ascontiguousarray(np.asarray(rel_table, f32))
    w["femb_wt"] = np.ascontiguousarray(np.asarray(femb_w, f32).T)
    w["femb_b_rep"] = np.ascontiguousarray(
        np.tile(np.asarray(femb_b, f32)[None, :], (128, 1)))
    for i, (gw, gas, gad, gb) in enumerate(
        [(gat1_w, gat1_asrc, gat1_adst, gat1_b),
         (gat2_w, gat2_asrc, gat2_adst, gat2_b)], start=1
    ):
        gw = np.asarray(gw, f32)
        aug = np.concatenate(
            [gw.T, (gw.T @ np.asarray(gas, f32))[:, None],
             (gw.T @ np.asarray(gad, f32))[:, None]], axis=1)
        w[f"waug{i}"] = np.ascontiguousarray(aug)  # [Din, 130]
        w[f"asrc{i}_rep"] = np.ascontiguousarray(
            np.tile(np.asarray(gas, f32)[None, :], (128, 1)))
        w[f"b{i}_rep"] = np.ascontiguousarray(
            np.tile(np.asarray(gb, f32)[None, :], (128, 1)))
    w["ident"] = np.eye(128, dtype=f32)
    return w


# ------------------------------------------------------- numpy device model --


def _sim_gather(table, idx_packed, num, elem):
    arr = idx_packed[:16]
    vals = arr.T.reshape(-1)[:num].astype(np.int64)
    rows = table[vals]
    return rows.reshape(num // 128, 128, elem).transpose(1, 0, 2)


def _fam_off(fam, t):
    oA = 8 * sum(fam["cA"][:t])
    oB = 8 * sum(fam["cB"][:t])
    oM = sum(fam["cA"][i] + fam["cB"][i] for i in range(t))
    return oA, oB, oM


def simulate(plan, weights, inputs):
    """Numpy mirror of the device program (validates the planner)."""
    f32 = np.float32
    attr_proj = (np.asarray(inputs["attr_table"], f32) @ weights["femb_wt"]
                 + weights["femb_b_rep"][0])
    proj_pad = np.zeros((NTOT, DE), f32)
    for c in range(NCORE):
        proj_pad[c * NPAD : c * NPAD + SHARD] = \
            attr_proj[c * SHARD : (c + 1) * SHARD]
    rel_proj = (np.asarray(inputs["rel_table"], f32) @ weights["femb_wt"]
                + weights["femb_b_rep"][0])
    rel_comb = np.concatenate([rel_proj, weights["rel_rows"]], axis=1)
    tab_A, tab_B = proj_pad[:SPLIT], proj_pad[SPLIT:]

    def softmax_gather(tabA, tabB, fam, core, t, query_fn, extra=None,
                       lrelu=False):
        cA, cB = fam["cA"][t], fam["cB"][t]
        oA, oB, oM = _fam_off(fam, t)
        gA = _sim_gather(tabA, fam["idxA"][core][:, oA : oA + 8 * cA],
                         128 * cA, DE)
        gB = _sim_gather(tabB, fam["idxB"][core][:, oB : oB + 8 * cB],
                         128 * cB, DE)
        mask = fam["mask"][core][:, oM : oM + cA + cB]
        G = np.concatenate([gA, gB], axis=1)
        s = (G * query_fn()).sum(-1)
        if extra is not None:
            s = s + extra
        s = s + mask
        if lrelu:
            s = np.maximum(s, NEG_SLOPE * s)
        m = s.max(axis=1, keepdims=True)
        ex = np.exp(s - m)
        return (G * ex[:, :, None]).sum(axis=1) / ex.sum(axis=1, keepdims=True)

    triple = np.zeros((NCORE, NPAD, 256), f32)
    for c in range(NCORE):
        for t in range(NTILE):
            rid = plan["ridx"][c][:16, 8 * t : 8 * t + 8].T.reshape(-1)[:128]
            rc = rel_comb[rid.astype(np.int64)]
            rp, re = rc[:, :128], rc[:, 128:]
            he = softmax_gather(tab_A, tab_B, plan["fam_h"], c, t,
                                lambda: rp[:, None, :])
            te = softmax_gather(tab_A, tab_B, plan["fam_t"], c, t,
                                lambda: rp[:, None, :])
            triple[c, t * 128 : (t + 1) * 128, :128] = he + te
            triple[c, t * 128 : (t + 1) * 128, 128:] = re

    def gat(x_all, waug, asrc_rep, b_rep, layer):
        h = x_all.reshape(NTOT, -1) @ waug
        hF = np.ascontiguousarray(h[:, :128])
        ad = h[:, 129]
        out = np.zeros((NCORE, NPAD, 128), f32)
        fam = plan["fam_e"]
        for c in range(NCORE):
            for g_i in range(NTILE):
                sl = slice(c * NPAD + g_i * 128, c * NPAD + (g_i + 1) * 128)
                agg = softmax_gather(
                    hF[:SPLIT], hF[SPLIT:], fam, c, g_i,
                    lambda: asrc_rep[0][None, None, :],
                    extra=ad[sl][:, None], lrelu=True)
                out[c, g_i * 128 : (g_i + 1) * 128] = agg + b_rep[0][None, :]
        return out

    x1 = gat(triple, weights["waug1"], weights["asrc1_rep"],
             weights["b1_rep"], 1)
    x2 = gat(x1, weights["waug2"], weights["asrc2_rep"], weights["b2_rep"], 2)
    return x2.reshape(NCORE, NPAD, 128)[plan["core_of"], plan["local_of"]]


# ------------------------------------------------------------ bass program --


def build_bass(plan):
    import copy as _copy
    import concourse.bass as bass
    import concourse.bacc as bacc
    import concourse.mybir as mb
    from contextlib import ExitStack

    F32 = mb.dt.float32
    I16 = mb.dt.int16
    fam_h, fam_t, fam_e = plan["fam_h"], plan["fam_t"], plan["fam_e"]

    nc = bacc.Bacc(target_bir_lowering=False, debug=True)

    def par(name, shape, dt=F32, out=False):
        return nc.declare_dram_parameter(name, list(shape), dt, isOutput=out)

    attr_tT = par("attr_tT", [128, N])
    rel_tT = par("rel_tT", [128, NREL])
    rel_rows = par("rel_rows", [NREL, 128])
    femb_wt = par("femb_wt", [128, 128])
    femb_b_rep = par("femb_b_rep", [128, 128])
    waug1 = par("waug1", [256, 130])
    waug2 = par("waug2", [128, 130])
    asrc1_rep = par("asrc1_rep", [128, 128])
    asrc2_rep = par("asrc2_rep", [128, 128])
    b1_rep = par("b1_rep", [128, 128])
    b2_rep = par("b2_rep", [128, 128])
    ident = par("ident", [128, 128])
    ridx_p = par("ridx", list(plan["ridx"][0].shape), I16)
    famp = {}
    for nm, fam in (("h", fam_h), ("t", fam_t), ("e", fam_e)):
        famp[nm] = dict(
            idxA=par(f"{nm}_idxA", list(fam["idxA"][0].shape), I16),
            idxB=par(f"{nm}_idxB", list(fam["idxB"][0].shape), I16),
            mask=par(f"{nm}_mask", list(fam["mask"][0].shape)),
        )
    out_ext = par("out", [NPAD, 128], out=True)

    proj_own = nc.dram_tensor("proj_own", [NPAD, 128], F32)
    d_attr = nc.dram_tensor("d_attr", [NTOT, 128], F32, addr_space="Shared")
    d_rel = nc.dram_tensor("d_rel", [NREL, 256], F32)
    triple = nc.dram_tensor("triple", [NPAD, 256], F32)
    h_own = nc.dram_tensor("h_own", [NPAD, 128], F32)
    d_h = nc.dram_tensor("d_h", [NTOT, 128], F32, addr_space="Shared")
    x2_own = nc.dram_tensor("x2_own", [NPAD, 128], F32)
    h2_own = nc.dram_tensor("h2_own", [NPAD, 128], F32)
    d_h2 = nc.dram_tensor("d_h2", [NTOT, 128], F32, addr_space="Shared")

    cmax = {
        "hA": max(fam_h["cA"]), "hB": max(fam_h["cB"]),
        "tA": max(fam_t["cA"]), "tB": max(fam_t["cB"]),
        "eA": max(fam_e["cA"]), "eB": max(fam_e["cB"]),
    }
    cmb_max = max(cmax["hA"] + cmax["hB"], cmax["tA"] + cmax["tB"],
                  cmax["eA"] + cmax["eB"])
    wcols = max(cmax.values()) * 128

    st = ExitStack()

    def sb(name, shape, dt=F32):
        return st.enter_context(nc.sbuf_tensor(name, list(shape), dt))

    def psum(name, shape):
        return st.enter_context(nc.psum_tensor(name, list(shape), F32))

    s_fembwt = sb("s_fembwt", [128, 128])
    s_femb_b = sb("s_femb_b", [128, 128])
    s_waug1 = sb("s_waug1", [128, 260])
    s_waug2 = sb("s_waug2", [128, 130])
    s_asrc = [sb("s_asrc1", [128, 128]), sb("s_asrc2", [128, 128])]
    s_bias = [sb("s_b1", [128, 128]), sb("s_b2", [128, 128])]
    s_ident = sb("s_ident", [128, 128])
    s_ridx = sb("s_ridx", [128, 8 * NTILE], I16)
    s_ad = [sb("s_ad1", [128, NTILE]), sb("s_ad2", [128, NTILE])]
    s_at = [sb(f"s_at{i}", [128, 128]) for i in range(2)]
    s_proj = [sb(f"s_proj{i}", [128, 128]) for i in range(2)]
    s_rel = [sb(f"s_rel{i}", [128, 256]) for i in range(2)]
    gbuf = {k: [sb(f"s_g{k}{i}", [128, cmax[k] * 128]) for i in range(2)]
            for k in cmax}
    ibuf = {k: [sb(f"s_i{k}{i}", [128, 8 * cmax[k]], I16) for i in range(2)]
            for k in cmax}
    mbuf = {k: [sb(f"s_m{k}{i}", [128, cmax[k + "A"] + cmax[k + "B"]])
                for i in range(2)] for k in ("h", "t", "e")}
    s_w1 = sb("s_w1", [128, wcols])
    s_sc = sb("s_sc", [128, cmb_max])
    s_ex = sb("s_ex", [128, cmb_max])
    s_red = sb("s_red", [128, 4])
    s_acc = sb("s_acc", [128, 128])
    s_acc2 = sb("s_acc2", [128, 128])
    s_emb = [sb("s_embh", [128, 128]), sb("s_embt", [128, 128])]
    s_x = [sb(f"s_x{i}", [128, 256]) for i in range(2)]
    s_xT = [sb(f"s_xT{i}", [128, 256]) for i in range(2)]
    s_h = [sb(f"s_h{i}", [128, 128]) for i in range(2)]
    s_o = [sb(f"s_o{i}", [128, 128]) for i in range(2)]
    s_z = sb("s_z", [128, 128])
    p_mm = [psum(f"p_mm{i}", [128, 130]) for i in range(2)]
    p_tr = [psum(f"p_tr{i}", [128, 128]) for i in range(2)]

    # ---------------- scheduling framework
    # DMA semaphores are split by purpose and tile parity so that every
    # wait covers the complete already-issued increment set on its sem
    # (race-detector-clean); compute sems (pe/act/dve/cc) update in issue
    # order and use plain cumulative counts.
    ENGS = ("gpsimd", "sync", "vector", "scalar", "tensor")
    SEMS = ("w", "p0a", "p0b", "ixa", "ixb", "gta", "gtb",
            "twa", "twb", "xa", "xb", "hwa", "hwb", "owa", "owb",
            "pe", "act", "dve", "cc", "gp", "msa", "msb", "pad")
    regs = {}
    ops = {e: [] for e in ENGS}
    cnt = {s: 0 for s in SEMS}
    last_wait = {e: {} for e in ENGS}

    def add(eng, emit, waits=(), inc=None):
        # same-engine pipelining can reorder element accesses: serialize
        # vector/scalar streams against themselves via their own sem.
        if eng == "vector":
            waits = list(waits) + [("dve", cnt["dve"])]
        elif eng == "scalar":
            waits = list(waits) + [("act", cnt["act"])]
        w = []
        for s_name, val in waits:
            if val <= 0 or last_wait[eng].get(s_name, -1) >= val:
                continue
            last_wait[eng][s_name] = val
            w.append((s_name, val))
        ops[eng].append((emit, tuple(w), inc))
        if inc:
            cnt[inc[0]] += inc[1]
        return dict(cnt)

    def pt(base, t):
        return base + ("a" if t % 2 == 0 else "b")

    def view_cf(buf_ap, c):      # [128, c*128] -> [128, c, 128]
        return buf_ap.rearrange("p (c f) -> p c f", f=128)

    def rep_mid(vec_ap, c):      # [128, 128] -> [128, c, 128] (0-step mid)
        return vec_ap.unsqueeze(1).broadcast_to([vec_ap.shape[0], c, 128])

    def exp_inner(sc_ap, c):     # [128, c] -> [128, c, 128] (0-step inner)
        return sc_ap.unsqueeze(2).broadcast_to([sc_ap.shape[0], c, 128])

    def jview(buf_ap, c):        # [128, c*128] -> [128, 128, c] (j innermost)
        return buf_ap.rearrange("p (c f) -> p c f", f=128).transpose([0, 2, 1])

    # ---------------- phase W: constants
    for dst, srcp in ((s_fembwt, femb_wt), (s_femb_b, femb_b_rep),
                      (s_waug2, waug2), (s_asrc[0], asrc1_rep),
                      (s_asrc[1], asrc2_rep), (s_bias[0], b1_rep),
                      (s_bias[1], b2_rep), (s_ident, ident), (s_ridx, ridx_p)):
        add("sync", lambda s, d=dst, so=srcp: s.dma_start(
            out=d[:, :], in_=so[:, :]), inc=("w", 16))
    add("sync", lambda s: s.dma_start(out=s_waug1[:, 0:130],
                                      in_=waug1[0:128, :]), inc=("w", 16))
    add("sync", lambda s: s.dma_start(out=s_waug1[:, 130:260],
                                      in_=waug1[128:256, :]), inc=("w", 16))
    W = cnt["w"]

    # ---------------- phase 0: table projections
    def proj_rows(src_cols, n_rows, out_dst, marks):
        ntl = (n_rows + 127) // 128
        for t in range(ntl):
            b = t % 2
            m = min(128, n_rows - t * 128)
            c0 = t * 128
            snap = add("sync", lambda s, b=b, c0=c0, m=m, sc=src_cols:
                       s.dma_start(out=s_at[b][:, 0:m],
                                   in_=sc[:, c0 : c0 + m]),
                       waits=[("pe", marks.get(("pe", b), 0))],
                       inc=(pt("p0", t), 16))
            snap = add("tensor", lambda te, b=b, m=m: te.matmul(
                p_tr[b][0:m, :], s_at[b][:, 0:m], s_fembwt[:, :],
                start=True, stop=True),
                waits=[(pt("p0", t), snap[pt("p0", t)]), ("w", W),
                       ("dve", marks.get(("dve", b), 0))],
                inc=("pe", 1))
            marks[("pe", b)] = snap["pe"]
            ms = pt("ms", t)
            snap = add("vector", lambda v, b=b, m=m: v.tensor_tensor(
                out=s_proj[b][0:m, :], in0=p_tr[b][0:m, :],
                in1=s_femb_b[0:m, :], op=mb.AluOpType.add),
                waits=[("pe", snap["pe"]), ("w", W),
                       (ms, marks.get(("ms", b), 0))],
                inc=("dve", 1))
            marks[("dve", b)] = snap["dve"]
            snap = add("gpsimd", lambda g, b=b, c0=c0, m=m, od=out_dst:
                       g.dma_start(out=od(c0, m), in_=s_proj[b][0:m, :]),
                       waits=[("dve", snap["dve"])], inc=(ms, 16))
            marks[("ms", b)] = snap[ms]
        return marks

    marks = proj_rows(attr_tT, SHARD,
                      lambda c0, m: proj_own[c0 : c0 + m, :], {})
    snap = add("gpsimd", lambda g: g.memset(s_z[:, :], 0.0), inc=("gp", 1))
    add("gpsimd", lambda g: g.dma_start(
        out=proj_own[SHARD:NPAD, :], in_=s_z[0 : NPAD - SHARD, :]),
        waits=[("gp", snap["gp"])], inc=("pad", 16))
    marks = proj_rows(rel_tT, NREL,
                      lambda c0, m: d_rel[c0 : c0 + m, 0:128], marks)
    add("gpsimd", lambda g: g.dma_start(out=d_rel[:, 128:256],
                                        in_=rel_rows[:, :]), inc=("pad", 16))
    MSA, MSB, GP = cnt["msa"], cnt["msb"], cnt["pad"]

    snap = add("gpsimd", lambda g: g.collective_compute(
        "AllGather", mb.AluOpType.bypass,
        replica_groups=[list(range(NCORE))],
        ins=[proj_own[:, :]], outs=[d_attr[:, :]]),
        waits=[("msa", MSA), ("msb", MSB), ("pad", GP)], inc=("cc", 1))
    cc_attr = snap["cc"]

    # ---------------- families: offsets
    offs = {"h": [_fam_off(fam_h, t) for t in range(NTILE + 1)],
            "t": [_fam_off(fam_t, t) for t in range(NTILE + 1)],
            "e": [_fam_off(fam_e, t) for t in range(NTILE + 1)]}

    def issue_idx(nm, fam, t, b, reuse_dve, reuse_gt):
        oA, oB, oM = offs[nm][t]
        cA, cB = fam["cA"][t], fam["cB"][t]
        pars = famp[nm]
        iA, iB = ibuf[nm + "A"][b], ibuf[nm + "B"][b]
        mB = mbuf[nm][b]
        ix, gt = pt("ix", t), pt("gt", t)
        add("sync", lambda s, iA=iA, oA=oA, cA=cA, pars=pars: s.dma_start(
            out=iA[:, 0 : 8 * cA], in_=pars["idxA"][:, oA : oA + 8 * cA]),
            waits=[(gt, reuse_gt), ("w", W)], inc=(ix, 16))
        add("sync", lambda s, iB=iB, oB=oB, cB=cB, pars=pars: s.dma_start(
            out=iB[:, 0 : 8 * cB], in_=pars["idxB"][:, oB : oB + 8 * cB]),
            inc=(ix, 16))
        snap = add("sync", lambda s, mB=mB, oM=oM, cc2=cA + cB, pars=pars:
                   s.dma_start(out=mB[:, 0 : cc2],
                               in_=pars["mask"][:, oM : oM + cc2]),
                   waits=[("dve", reuse_dve)], inc=(ix, 16))
        return snap

    def issue_gat(nm, fam, t, b, tabA, tabB, ix_snap, reuse_dve, extra_gw=()):
        cA, cB = fam["cA"][t], fam["cB"][t]
        bA, bB = gbuf[nm + "A"][b], gbuf[nm + "B"][b]
        iA, iB = ibuf[nm + "A"][b], ibuf[nm + "B"][b]
        ix, gt = pt("ix", t), pt("gt", t)
        gw = ([(ix, ix_snap[ix]), ("dve", reuse_dve)] + list(extra_gw))

        GCHUNK = 16  # ucode packet/ring limits: stay <= 2048 idx per gather

        def _gather(g, buf, ib, c0, c1, tab):
            g.reg_mov(regs["g"], 128 * (c1 - c0))
            return g.dma_gather(
                out_ap=view_cf(buf[:, c0 * 128 : c1 * 128], c1 - c0),
                in_ap=tab, idxs_ap=ib[:, 8 * c0 : 8 * c1],
                num_idxs=128 * (c1 - c0), num_idxs_reg=regs["g"],
                elem_size=128, single_packet=False)

        snap = None
        for buf, ib, cX, tab in ((bA, iA, cA, tabA), (bB, iB, cB, tabB)):
            for c0 in range(0, cX, GCHUNK):
                c1 = min(c0 + GCHUNK, cX)
                snap = add("gpsimd",
                           lambda g, buf=buf, ib=ib, c0=c0, c1=c1, tab=tab:
                           _gather(g, buf, ib, c0, c1, tab),
                           waits=gw, inc=(gt, 16))
        return snap, cA, cB

    def attention(nm, cA, cB, b, query_ap_fn, first_waits, extra_ap=None,
                  lrelu=False):
        c = cA + cB
        bufs = (gbuf[nm + "A"][b], gbuf[nm + "B"][b])
        mask = mbuf[nm][b]
        for i, (cX, buf, o0) in enumerate(((cA, bufs[0], 0),
                                           (cB, bufs[1], cA))):
            q_ap = query_ap_fn(cX)
            add("vector", lambda v, cX=cX, buf=buf, q=q_ap: v.tensor_tensor(
                out=view_cf(s_w1[:, 0 : cX * 128], cX),
                in0=view_cf(buf[:, 0 : cX * 128], cX), in1=q,
                op=mb.AluOpType.mult),
                waits=first_waits if i == 0 else (), inc=("dve", 1))
            add("vector", lambda v, cX=cX, o0=o0: v.tensor_reduce(
                out=s_sc[:, o0 : o0 + cX],
                in_=view_cf(s_w1[:, 0 : cX * 128], cX),
                axis=mb.AxisListType.X, op=mb.AluOpType.add), inc=("dve", 1))
        if extra_ap is not None:
            add("vector", lambda v, e=extra_ap, c=c: v.tensor_scalar_add(
                s_sc[:, 0:c], s_sc[:, 0:c], e), inc=("dve", 1))
        add("vector", lambda v, c=c, mask=mask: v.tensor_tensor(
            out=s_sc[:, 0:c], in0=s_sc[:, 0:c], in1=mask[:, 0:c],
            op=mb.AluOpType.add), inc=("dve", 1))
        if lrelu:
            add("vector", lambda v, c=c: v.tensor_scalar_mul(
                s_ex[:, 0:c], s_sc[:, 0:c], NEG_SLOPE), inc=("dve", 1))
            add("vector", lambda v, c=c: v.tensor_tensor(
                out=s_sc[:, 0:c], in0=s_sc[:, 0:c], in1=s_ex[:, 0:c],
                op=mb.AluOpType.max), inc=("dve", 1))
        snap = add("vector", lambda v, c=c: v.tensor_reduce(
            out=s_red[:, 0:1], in_=s_sc[:, 0:c], axis=mb.AxisListType.X,
            op=mb.AluOpType.max, negate=True), inc=("dve", 1))
        snap = add("scalar", lambda sc, c=c: sc.activation(
            out=s_ex[:, 0:c], in_=s_sc[:, 0:c],
            func=mb.ActivationFunctionType.Exp,
            bias=s_red[:, 0:1], accum_out=s_red[:, 1:2]),
            waits=[("dve", snap["dve"])], inc=("act", 1))
        snap = add("vector", lambda v: v.reciprocal(s_red[:, 2:3],
                                                    s_red[:, 1:2]),
                   waits=[("act", snap["act"])], inc=("dve", 1))
        for i, (cX, buf, o0) in enumerate(((cA, bufs[0], 0),
                                           (cB, bufs[1], cA))):
            acc = s_acc if i == 0 else s_acc2
            add("vector", lambda v, cX=cX, buf=buf, o0=o0: v.tensor_tensor(
                out=view_cf(s_w1[:, 0 : cX * 128], cX),
                in0=view_cf(buf[:, 0 : cX * 128], cX),
                in1=exp_inner(s_ex[:, o0 : o0 + cX], cX),
                op=mb.AluOpType.mult), inc=("dve", 1))
            add("vector", lambda v, cX=cX, acc=acc: v.tensor_reduce(
                out=acc[:, :], in_=jview(s_w1[:, 0 : cX * 128], cX),
                axis=mb.AxisListType.X, op=mb.AluOpType.add), inc=("dve", 1))
        snap = add("vector", lambda v: v.tensor_tensor(
            out=s_acc[:, :], in0=s_acc[:, :], in1=s_acc2[:, :],
            op=mb.AluOpType.add), inc=("dve", 1))
        return snap

    import os as _os
    _STOP = int(_os.environ.get("BUILD_STOP", "9"))
    if _STOP < 1:
        NT1 = 0
    else:
        NT1 = NTILE
    # ---------------- phase 1: entity embedding
    emb_dve_done, emb_gt_done, emb_tw = {}, {}, {}
    for t in range(NT1):
        b = t % 2
        gt, tw = pt("gt", t), pt("tw", t)
        reuse_d = emb_dve_done.get(t - 2, 0)
        reuse_gt = emb_gt_done.get(t - 2, 0)

        def _relgather(g, t, b):
            g.reg_mov(regs["g"], 128)
            return g.dma_gather(
                out_ap=s_rel[b][:, :].unsqueeze(1),
                in_ap=d_rel[:, :], idxs_ap=s_ridx[:, 8 * t : 8 * t + 8],
                num_idxs=128, num_idxs_reg=regs["g"], elem_size=256)
        snap = add("gpsimd", lambda g, t=t, b=b: _relgather(g, t, b),
                   waits=[("cc", cc_attr), ("w", W), ("msa", MSA),
                          ("msb", MSB), ("pad", GP), ("dve", reuse_d),
                          (tw, emb_tw.get(t - 2, 0))],
                   inc=(gt, 16))
        issue_idx("h", fam_h, t, b, reuse_d, reuse_gt)
        ix_snap = issue_idx("t", fam_t, t, b, reuse_d, reuse_gt)
        snap, cAh, cBh = issue_gat(
            "h", fam_h, t, b, d_attr[0:SPLIT, :], d_attr[SPLIT:NTOT, :],
            ix_snap, reuse_d, [("cc", cc_attr)])
        snap, cAt, cBt = issue_gat(
            "t", fam_t, t, b, d_attr[0:SPLIT, :], d_attr[SPLIT:NTOT, :],
            ix_snap, reuse_d)
        emb_gt_done[t] = snap[gt]
        gw = [(gt, snap[gt])]
        rp_fn = lambda cX, b=b: rep_mid(s_rel[b][:, 0:128], cX)
        attention("h", cAh, cBh, b, rp_fn, gw)
        add("vector", lambda v: v.tensor_scalar_mul(
            s_emb[0][:, :], s_acc[:, :], s_red[:, 2:3]), inc=("dve", 1))
        attention("t", cAt, cBt, b, rp_fn, ())
        add("vector", lambda v: v.tensor_scalar_mul(
            s_emb[1][:, :], s_acc[:, :], s_red[:, 2:3]), inc=("dve", 1))
        snap = add("vector", lambda v, b=b: v.tensor_tensor(
            out=s_o[b][:, :], in0=s_emb[0][:, :], in1=s_emb[1][:, :],
            op=mb.AluOpType.add),
            waits=[(pt("ow", t), 0)], inc=("dve", 1))
        emb_dve_done[t] = snap["dve"]
        add("gpsimd", lambda g, t=t, b=b: g.dma_start(
            out=triple[128 * t : 128 * (t + 1), 0:128], in_=s_o[b][:, :]),
            waits=[("dve", snap["dve"])], inc=(tw, 16))
        snap = add("gpsimd", lambda g, t=t, b=b: g.dma_start(
            out=triple[128 * t : 128 * (t + 1), 128:256],
            in_=s_rel[b][:, 128:256]), inc=(tw, 16))
        emb_tw[t] = snap[tw]
    TWA, TWB = cnt["twa"], cnt["twb"]
    if _STOP < 2:
        NTILE_MM = 0
    else:
        NTILE_MM = NTILE

    # ---------------- GAT dense matmuls (phase-barriered on inputs)
    def gat_matmul(layer, x_src, nchunks, h_dst, in_waits):
        mm_act, h_hw = {}, {}
        waug = s_waug1 if layer == 1 else s_waug2
        for t in range(NTILE_MM):
            b = t % 2
            x, hw = pt("x", t), pt("hw", t)
            snap = add("sync", lambda s, t=t, b=b, nk=nchunks: s.dma_start(
                out=s_x[b][:, 0 : 128 * nk], in_=x_src(t)),
                waits=list(in_waits) + [("act", mm_act.get(t - 2, 0))],
                inc=(x, 16))
            sd = snap[x]
            a_snap = 0
            for k in range(nchunks):
                snap = add("tensor", lambda te, b=b, k=k: te.transpose(
                    out=p_tr[b][:, :], in_=s_x[b][:, 128 * k : 128 * (k + 1)],
                    identity=s_ident[:, :]),
                    waits=[(x, sd), ("act", a_snap), ("w", W)], inc=("pe", 1))
                snap = add("scalar", lambda sc, b=b, k=k: sc.activation(
                    out=s_xT[b][:, 128 * k : 128 * (k + 1)],
                    in_=p_tr[b][:, :], func=mb.ActivationFunctionType.Copy),
                    waits=[("pe", snap["pe"])], inc=("act", 1))
                a_snap = snap["act"]
            for k in range(nchunks):
                snap = add("tensor", lambda te, b=b, k=k, waug=waug,
                           nk=nchunks: te.matmul(
                    p_mm[b][:, :], s_xT[b][:, 128 * k : 128 * (k + 1)],
                    waug[:, 130 * k : 130 * (k + 1)],
                    start=(k == 0), stop=(k == nk - 1)),
                    waits=[("act", a_snap)], inc=("pe", 1))
            snap = add("scalar", lambda sc, b=b: sc.activation(
                out=s_h[b][:, :], in_=p_mm[b][:, 0:128],
                func=mb.ActivationFunctionType.Copy),
                waits=[("pe", snap["pe"]), (hw, h_hw.get(t - 2, 0))],
                inc=("act", 1))
            snap = add("scalar", lambda sc, b=b, t=t, lay=layer: sc.activation(
                out=s_ad[lay - 1][:, t : t + 1], in_=p_mm[b][:, 129:130],
                func=mb.ActivationFunctionType.Copy), inc=("act", 1))
            mm_act[t] = snap["act"]
            snap = add("gpsimd", lambda g, t=t, b=b, hd=h_dst: g.dma_start(
                out=hd[128 * t : 128 * (t + 1), :], in_=s_h[b][:, :]),
                waits=[("act", snap["act"])], inc=(hw, 16))
            h_hw[t] = snap[hw]
        return dict(cnt)

    mm1 = gat_matmul(1, lambda t: triple[128 * t : 128 * (t + 1), :], 2,
                     h_own, [("twa", TWA), ("twb", TWB)])
    if _STOP >= 3:
        snap = add("gpsimd", lambda g: g.collective_compute(
            "AllGather", mb.AluOpType.bypass,
            replica_groups=[list(range(NCORE))],
            ins=[h_own[:, :]], outs=[d_h[:, :]]),
            waits=[("hwa", mm1["hwa"]), ("hwb", mm1["hwb"])], inc=("cc", 1))
        cc_h1 = snap["cc"]
    else:
        cc_h1 = 0

    # ---------------- edge phases
    def edge_phase(layer, d_tab, out_dst, cc_need):
        ed_done, ed_gt, ed_ow = {}, {}, {}
        bias = s_bias[layer - 1]
        asr = s_asrc[layer - 1]
        ad_col = s_ad[layer - 1]
        for g_i in range(NTILE if _STOP >= 4 else 0):
            b = g_i % 2
            gt, ow = pt("gt", g_i), pt("ow", g_i)
            reuse_d = ed_done.get(g_i - 2, 0)
            reuse_gt = ed_gt.get(g_i - 2, 0)
            ix_snap = issue_idx("e", fam_e, g_i, b, reuse_d, reuse_gt)
            snap, cA, cB = issue_gat(
                "e", fam_e, g_i, b, d_tab[0:SPLIT, :], d_tab[SPLIT:NTOT, :],
                ix_snap, reuse_d, [("cc", cc_need)])
            ed_gt[g_i] = snap[gt]
            gw = [(gt, snap[gt])]
            q_fn = lambda cX, asr=asr: rep_mid(asr[:, 0:128], cX)
            attention("e", cA, cB, b, q_fn, gw,
                      extra_ap=ad_col[:, g_i : g_i + 1], lrelu=True)
            snap = add("vector", lambda v, b=b: v.tensor_scalar_mul(
                s_o[b][:, :], s_acc[:, :], s_red[:, 2:3]),
                waits=[(ow, ed_ow.get(g_i - 2, 0))], inc=("dve", 1))
            snap = add("vector", lambda v, b=b, bias=bias: v.tensor_tensor(
                out=s_o[b][:, :], in0=s_o[b][:, :], in1=bias[:, :],
                op=mb.AluOpType.add), inc=("dve", 1))
            ed_done[g_i] = snap["dve"]
            snap = add("gpsimd", lambda g, g_i=g_i, b=b, od=out_dst:
                       g.dma_start(
                           out=od[128 * g_i : 128 * (g_i + 1), :],
                           in_=s_o[b][:, :]),
                       waits=[("dve", snap["dve"])], inc=(ow, 16))
            ed_ow[g_i] = snap[ow]
        return dict(cnt)

    e1 = edge_phase(1, d_h, x2_own, cc_h1)
    if _STOP >= 5:
        mm2 = gat_matmul(2, lambda t: x2_own[128 * t : 128 * (t + 1), :], 1,
                         h2_own, [("owa", e1["owa"]), ("owb", e1["owb"])])
        snap = add("gpsimd", lambda g: g.collective_compute(
            "AllGather", mb.AluOpType.bypass,
            replica_groups=[list(range(NCORE))],
            ins=[h2_own[:, :]], outs=[d_h2[:, :]]),
            waits=[("hwa", mm2["hwa"]), ("hwb", mm2["hwb"])], inc=("cc", 1))
        cc_h2 = snap["cc"]
        if _STOP >= 6:
            edge_phase(2, d_h2, out_ext, cc_h2)

    if _STOP < 9:
        snap0 = add("gpsimd", lambda g: g.dma_start(
            out=out_ext[0:128, :], in_=s_z[:, :]), inc=("pad", 16))
    final = dict(cnt)
    import os
    if os.environ.get("BASS_PRINT_SEMS"):
        print("FINAL SEM COUNTS:", final)

    # ---------------- emit
    with ExitStack() as es:
        block = es.enter_context(nc.Block())
        sems = {s_name: es.enter_context(nc.semaphore(f"sem_{s_name}"))
                for s_name in SEMS}

        def make_body(eng_name):
            def body(eng):
                if eng_name == "gpsimd":
                    regs["g"] = es.enter_context(eng.register("gnum"))
                for emit, waits, inc in ops[eng_name]:
                    for s_name, val in waits:
                        eng.wait_ge(sems[s_name], val)
                    inst = emit(eng)
                    if inc is not None and inst is not None:
                        inst.then_inc(sems[inc[0]], inc[1])
                if eng_name == "gpsimd":
                    for s_name in SEMS:
                        if s_name != "cc" and final[s_name] > 0:
                            eng.wait_ge(sems[s_name], final[s_name])
            return body

        block.gpsimd(make_body("gpsimd"))
        block.sync(make_body("sync"))
        block.vector(make_body("vector"))
        block.scalar(make_body("scalar"))
        block.tensor(make_body("tensor"))

    nc.compile()
    st.close()
    return nc


# ---------------------------------------------------------------- kernel() --

_CACHE = {}


def _prepare(inputs):
    plan = make_plan(inputs["h_attributes"], inputs["t_attributes"],
                     inputs["r_idx"], inputs["edge_index"])
    weights = make_weights(
        inputs["attr_table"], inputs["rel_table"], inputs["femb_w"],
        inputs["femb_b"], inputs["gat1_w"], inputs["gat1_asrc"],
        inputs["gat1_adst"], inputs["gat1_b"], inputs["gat2_w"],
        inputs["gat2_asrc"], inputs["gat2_adst"], inputs["gat2_b"])
    in_maps = []
    for c in range(NCORE):
        m = dict(
            attr_tT=np.ascontiguousarray(
                np.roll(weights["attr_tT"], -c * SHARD, axis=1)),
            rel_tT=weights["rel_tT"], rel_rows=weights["rel_rows"],
            femb_wt=weights["femb_wt"], femb_b_rep=weights["femb_b_rep"],
            waug1=weights["waug1"], waug2=weights["waug2"],
            asrc1_rep=weights["asrc1_rep"], asrc2_rep=weights["asrc2_rep"],
            b1_rep=weights["b1_rep"], b2_rep=weights["b2_rep"],
            ident=weights["ident"], ridx=plan["ridx"][c],
        )
        for nm in ("h", "t", "e"):
            fam = plan[f"fam_{nm}"]
            m[f"{nm}_idxA"] = fam["idxA"][c]
            m[f"{nm}_idxB"] = fam["idxB"][c]
            m[f"{nm}_mask"] = fam["mask"][c]
        in_maps.append(m)
    return plan, weights, in_maps


LAST_EXEC_NS = None


def kernel(**inputs):
    global LAST_EXEC_NS
    import os
    plan, weights, in_maps = _prepare(inputs)
    nc = build_bass(plan)
    from concourse.bass_utils import run_bass_kernel_spmd
    trace = bool(os.environ.get("BASS_KTRACE"))
    res = run_bass_kernel_spmd(nc, in_maps, list(range(NCORE)), trace=trace)
    if res.exec_time_ns is not None:
        LAST_EXEC_NS = res.exec_time_ns
    outs = np.stack([np.asarray(res.results[c]["out"]) for c in range(NCORE)])
    return np.ascontiguousarray(
        outs[plan["core_of"], plan["local_of"]].astype(np.float32))


def _chain_setup(nc, in_maps):
    """Shared staging for the chained-execution benchmark: returns a
    function make_fn(K) producing a jitted SPMD callable that runs the NEFF
    K times back-to-back (output buffers threaded through as data deps, so
    the K executions serialize on-device), plus the staged operands."""
    import jax
    from jax.sharding import Mesh, PartitionSpec, NamedSharding
    from concourse import bass2jax
    import concourse.mybir as mybir

    bass2jax.install_neuronx_cc_hook()
    partition_name = (nc.partition_id_tensor.name
                      if nc.partition_id_tensor else None)
    in_names, out_names, out_avals, zero_outs = [], [], [], []
    for alloc in nc.m.functions[0].allocations:
        if not isinstance(alloc, mybir.MemoryLocationSet):
            continue
        name = alloc.memorylocations[0].name
        if alloc.kind == "ExternalInput":
            if name != partition_name:
                in_names.append(name)
        elif alloc.kind == "ExternalOutput":
            shape = tuple(alloc.tensor_shape)
            dtype = mybir.dt.np(alloc.dtype)
            out_names.append(name)
            out_avals.append(jax.core.ShapedArray(shape, dtype))
            zero_outs.append(np.zeros(shape, dtype))
    n_params = len(in_names)
    all_names = list(in_names) + list(out_names)
    if partition_name is not None:
        all_names.append(partition_name)

    def make_fn(K):
        def _chain(*args):
            ins = list(args[:n_params])
            outs = list(args[n_params:])
            part = ([bass2jax.partition_id_tensor()]
                    if partition_name is not None else [])
            for _ in range(K):
                outs = list(bass2jax._bass_exec_p.bind(
                    *(ins + outs + part), out_avals=tuple(out_avals),
                    in_names=tuple(all_names), out_names=tuple(out_names),
                    lowering_input_output_aliases=(),
                    sim_require_finite=True, sim_require_nnan=True, nc=nc))
            return tuple(outs)

        from jax.experimental.shard_map import shard_map
        devices = jax.devices()[:NCORE]
        mesh = Mesh(np.asarray(devices), ("core",))
        n_outs = len(out_names)
        in_specs = (PartitionSpec("core"),) * (n_params + n_outs)
        out_specs = (PartitionSpec("core"),) * n_outs
        return jax.jit(shard_map(_chain, mesh=mesh, in_specs=in_specs,
                                 out_specs=out_specs, check_rep=False),
                       keep_unused=True), mesh

    maps = in_maps
    if nc.dbg_addr is not None:
        maps = [{**m, nc.dbg_addr.name: np.zeros((1, 2), np.uint32)}
                for m in maps]
    per_core = [[np.asarray(m[k]) for k in in_names[:n_params]]
                for m in maps]
    import jax
    devices = jax.devices()[:NCORE]
    mesh = Mesh(np.asarray(devices), ("core",))
    sh = NamedSharding(mesh, PartitionSpec("core"))
    concat_in = [jax.device_put(
        np.concatenate([per_core[c][i] for c in range(NCORE)], axis=0), sh)
        for i in range(n_params)]
    zglobal = [jax.device_put(
        np.zeros((NCORE * z.shape[0], *z.shape[1:]), z.dtype), sh)
        for z in zero_outs]
    return make_fn, concat_in, zglobal


def bench_chain(inputs, k_short=2, k_long=10, reps=5):
    """Measure marginal per-execution HW time: run chains of k_short and
    k_long back-to-back NEFF executions inside single dispatches; the slope
    (T_long - T_short) / (k_long - k_short) cancels dispatch overhead.
    Returns (marginal_s, dict of raw timings)."""
    import time
    plan, weights, in_maps = _prepare(inputs)
    nc = build_bass(plan)
    make_fn, concat_in, zglobal = _chain_setup(nc, in_maps)
    out = {}
    for k in (k_short, k_long):
        fn, _ = make_fn(k)
        times = []
        for it in range(reps + 1):
            t0 = time.perf_counter()
            res = fn(*concat_in, *zglobal)
            for o in res:
                o.block_until_ready()
            times.append(time.perf_counter() - t0)
        out[k] = times[1:]  # drop compile/warmup iter
    marginal = (min(out[k_long]) - min(out[k_short])) / (k_long - k_short)
    return marginal, out


def bench(n_iter=5, inputs=None):
    """Time repeated NEFF executions (inputs staged once; outputs donated
    fresh each iter). Returns (best_s, all_s)."""
    import time
    import jax
    import jax.numpy as jnp
    from jax.sharding import Mesh, PartitionSpec, NamedSharding
    plan, weights, in_maps = _prepare(inputs)
    nc = build_bass(plan)
    from concourse import bass2jax

    # replicate run_bass_via_pjrt, but keep the compiled callable
    import concourse.mybir as mybir
    bass2jax.install_neuronx_cc_hook()
    partition_name = (nc.partition_id_tensor.name
                      if nc.partition_id_tensor else None)
    in_names, out_names, out_avals, zero_outs = [], [], [], []
    for alloc in nc.m.functions[0].allocations:
        if not isinstance(alloc, mybir.MemoryLocationSet):
            continue
        name = alloc.memorylocations[0].name
        if alloc.kind == "ExternalInput":
            if name != partition_name:
                in_names.append(name)
        elif alloc.kind == "ExternalOutput":
            shape = tuple(alloc.tensor_shape)
            dtype = mybir.dt.np(alloc.dtype)
            out_names.append(name)
            out_avals.append(jax.core.ShapedArray(shape, dtype))
            zero_outs.append(np.zeros(shape, dtype))
    n_params = len(in_names)
    n_outs = len(out_avals)
    in_names.extend(out_names)
    if partition_name is not None:
        in_names.append(partition_name)
    donate = tuple(range(n_params, n_params + n_outs))

    def _body(*args):
        operands = list(args)
        if partition_name is not None:
            operands.append(bass2jax.partition_id_tensor())
        return tuple(bass2jax._bass_exec_p.bind(
            *operands, out_avals=tuple(out_avals), in_names=tuple(in_names),
            out_names=tuple(out_names), lowering_input_output_aliases=(),
            sim_require_finite=True, sim_require_nnan=True, nc=nc))

    from jax.experimental.shard_map import shard_map
    devices = jax.devices()[:NCORE]
    mesh = Mesh(np.asarray(devices), ("core",))
    in_specs = (PartitionSpec("core"),) * (n_params + n_outs)
    out_specs = (PartitionSpec("core"),) * len(out_names)
    fn = jax.jit(shard_map(_body, mesh=mesh, in_specs=in_specs,
                           out_specs=out_specs, check_rep=False),
                 donate_argnums=donate, keep_unused=True)
    if nc.dbg_addr is not None:
        in_maps = [{**m, nc.dbg_addr.name: np.zeros((1, 2), np.uint32)}
                   for m in in_maps]
    per_core = [[np.asarray(m[k]) for k in in_names[:n_params]]
                for m in in_maps]
    sh = NamedSharding(mesh, PartitionSpec("core"))
    concat_in = [jax.device_put(
        np.concatenate([per_core[c][i] for c in range(NCORE)], axis=0), sh)
        for i in range(n_params)]
    zglobal = [np.zeros((NCORE * z.shape[0], *z.shape[1:]), z.dtype)
               for z in zero_outs]
    times = []
    for it in range(n_iter):
        zs = [jax.device_put(z, sh) for z in zglobal]
        for z in zs:
            z.block_until_ready()
        t0 = time.perf_counter()
        outs = fn(*concat_in, *zs)
        for o in outs:
            o.block_until_ready()
        times.append(time.perf_counter() - t0)
    return min(times), times

